# revision 1
# baseline (speedup 1.0000x reference)
import os
import sys

if "/opt/trn_rl_repo" not in sys.path:
    sys.path.insert(0, "/opt/trn_rl_repo")

from contextlib import ExitStack

import numpy as np

import concourse.bass as bass
import concourse.tile as tile
from concourse import bacc, masks, mybir
from concourse.bass_utils import run_bass_kernel_spmd

F32 = mybir.dt.float32
I32 = mybir.dt.int32
AF = mybir.ActivationFunctionType
ALU = mybir.AluOpType
AX = mybir.AxisListType

N_CORES = 8
AGENT_SIZE = 64
NEIGH_SIZE = 64
NUM_HEAD = 4
MID_SIZE = 32
NK = 4
HM = NUM_HEAD * MID_SIZE
OUT = HM // 2

_BUILD_CACHE = {}


def _emit_chunk(ctx, tc, pools, aps, c0, P, has_bias):
    nc = tc.nc
    (inp, xt_ps, xt_sb, proj_ps, work, out_ps, const) = pools
    (agent, neigh, mask, y, wa, wn, wh, wo4, bias_abc, bias_o, ident) = aps

    ag = inp.tile([128, AGENT_SIZE], F32, tag="ag")
    nc.sync.dma_start(ag[:P], agent[c0 : c0 + P, :])
    nb = inp.tile([128, NK * NEIGH_SIZE], F32, tag="nb")
    nc.sync.dma_start(nb[:P], neigh[c0 : c0 + P, :])
    mk = inp.tile([128, NK], I32, tag="mk")
    nc.sync.dma_start(mk[:P], mask[c0 : c0 + P, :])

    t1 = xt_ps.tile([64, 512], F32, tag="t1")
    t2 = xt_ps.tile([64, 128], F32, tag="t2")
    nc.tensor.transpose(t1[:, 0:P], ag[:P, :], ident[:P, :P])
    for k in range(3):
        nc.tensor.transpose(
            t1[:, 128 * (k + 1) : 128 * (k + 1) + P],
            nb[:P, 64 * k : 64 * (k + 1)],
            ident[:P, :P],
        )
    nc.tensor.transpose(t2[:, 0:P], nb[:P, 64 * 3 : 64 * 4], ident[:P, :P])

    xt = xt_sb.tile([64, 640], F32, tag="xt")
    nc.scalar.activation(xt[:, 0:512], t1[:, :], AF.Copy)
    nc.scalar.activation(xt[:, 512:640], t2[:, :], AF.Copy)
    agT = lambda: xt[:, 0:P]
    nbT = lambda k: xt[:, 128 * (k + 1) : 128 * (k + 1) + P]

    pa = proj_ps.tile([128, 512], F32, tag="pa")
    pb = proj_ps.tile([128, 512], F32, tag="pb")
    pc = out_ps.tile([128, 320], F32, tag="pc")
    nc.tensor.matmul(pa[:P, 0:128], agT(), wa[:, :])
    for k in range(4):
        dst = pa[:P, 128 * (k + 1) : 128 * (k + 2)] if k < 3 else pb[:P, 0:128]
        nc.tensor.matmul(dst, nbT(k), wn[:, :])
    for k in range(4):
        dst = pb[:P, 128 * (k + 1) : 128 * (k + 2)] if k < 3 else pc[:P, 0:128]
        nc.tensor.matmul(dst, nbT(k), wh[:, :])

    a_psv = pa[:P, 0:128]
    nr_psv = [
        pa[:P, 128:256],
        pa[:P, 256:384],
        pa[:P, 384:512],
        pb[:P, 0:128],
    ]
    nh_psv = [
        pb[:P, 128:256],
        pb[:P, 256:384],
        pb[:P, 384:512],
        pc[:P, 0:128],
    ]

    ba = bias_abc
    a_r = work.tile([128, HM], F32, tag="a_r")
    nr_r = work.tile([128, 4 * HM], F32, tag="nr_r")
    nh_r = work.tile([128, 4 * HM], F32, tag="nh_r")
    if has_bias:
        nc.vector.tensor_tensor(a_psv, a_psv, ba[:P, 0:128], op=ALU.add)
        for k in range(4):
            nc.vector.tensor_tensor(nr_psv[k], nr_psv[k], ba[:P, 128:256], op=ALU.add)
            nc.vector.tensor_tensor(nh_psv[k], nh_psv[k], ba[:P, 256:384], op=ALU.add)
    nc.vector.tensor_scalar_max(a_r[:P], a_psv, 0.0)
    for k in range(4):
        dst_nr = nr_r[:P, 128 * k : 128 * (k + 1)]
        dst_nh = nh_r[:P, 128 * k : 128 * (k + 1)]
        if k < 3:
            nc.vector.tensor_scalar_max(dst_nr, nr_psv[k], 0.0)
            nc.scalar.activation(dst_nh, nh_psv[k], AF.Relu)
        else:
            nc.scalar.activation(dst_nr, nr_psv[k], AF.Relu)
            nc.scalar.activation(dst_nh, nh_psv[k], AF.Relu)

    att = work.tile([128, NUM_HEAD * NK], F32, tag="att")
    prod = work.tile([128, HM], F32, tag="prod")
    att_v = att[:P].rearrange("p (h k) -> p h k", k=NK)
    for k in range(4):
        nc.vector.tensor_tensor(
            prod[:P], a_r[:P], nr_r[:P, 128 * k : 128 * (k + 1)], op=ALU.mult
        )
        nc.vector.tensor_reduce(
            att_v[:, :, k],
            prod[:P].rearrange("p (h m) -> p h m", h=NUM_HEAD),
            axis=AX.X,
            op=ALU.add,
        )

    mkp = work.tile([128, NK], F32, tag="mkp")
    nc.vector.tensor_scalar_mul(mkp[:P], mk[:P], -1.0e8)
    attm = work.tile([128, NUM_HEAD * NK], F32, tag="attm")
    mkp_b = mkp[:P].unsqueeze(1).broadcast_to([P, NUM_HEAD, NK])
    nc.vector.tensor_tensor(
        attm[:P].rearrange("p (h k) -> p h k", k=NK), att_v, mkp_b, op=ALU.add
    )
    es = work.tile([128, NUM_HEAD * NK], F32, tag="es")
    nc.scalar.activation(es[:P], attm[:P], AF.Exp)
    s4 = work.tile([128, NUM_HEAD], F32, tag="s4")
    nc.vector.tensor_reduce(
        s4[:P], es[:P].rearrange("p (h k) -> p h k", k=NK), axis=AX.X, op=ALU.add
    )
    s4m = work.tile([128, NUM_HEAD], F32, tag="s4m")
    nc.vector.tensor_scalar_max(s4m[:P], s4[:P], 1.0e-30)
    r4 = work.tile([128, NUM_HEAD], F32, tag="r4")
    nc.vector.reciprocal(r4[:P], s4m[:P])
    score = work.tile([128, NUM_HEAD * NK], F32, tag="score")
    r4_b = r4[:P].unsqueeze(2).broadcast_to([P, NUM_HEAD, NK])
    nc.vector.tensor_tensor(
        score[:P].rearrange("p (h k) -> p h k", k=NK),
        es[:P].rearrange("p (h k) -> p h k", k=NK),
        r4_b,
        op=ALU.mult,
    )

    wk01 = work.tile([128, HM], F32, tag="wk01")
    wk23 = work.tile([128, HM], F32, tag="wk23")
    wkt = work.tile([128, HM], F32, tag="wkt")
    outacc = work.tile([128, HM], F32, tag="outacc")
    sc_v = score[:P].rearrange("p (h k) -> p h k", k=NK)

    def score_k(k):
        return sc_v[:, :, k].unsqueeze(2).broadcast_to([P, NUM_HEAD, MID_SIZE])

    def nh_k(k):
        return nh_r[:P, 128 * k : 128 * (k + 1)].rearrange(
            "p (h m) -> p h m", h=NUM_HEAD
        )

    ge = nc.gpsimd
    ge.tensor_tensor(
        wk01[:P].rearrange("p (h m) -> p h m", h=NUM_HEAD), score_k(0), nh_k(0), op=ALU.mult
    )
    ge.tensor_tensor(
        wkt[:P].rearrange("p (h m) -> p h m", h=NUM_HEAD), score_k(1), nh_k(1), op=ALU.mult
    )
    ge.tensor_tensor(wk01[:P], wk01[:P], wkt[:P], op=ALU.add)
    ge.tensor_tensor(
        wk23[:P].rearrange("p (h m) -> p h m", h=NUM_HEAD), score_k(2), nh_k(2), op=ALU.mult
    )
    ge.tensor_tensor(
        wkt[:P].rearrange("p (h m) -> p h m", h=NUM_HEAD), score_k(3), nh_k(3), op=ALU.mult
    )
    ge.tensor_tensor(wk23[:P], wk23[:P], wkt[:P], op=ALU.add)
    ge.tensor_tensor(outacc[:P], wk01[:P], wk23[:P], op=ALU.add)

    oT_ps = pc[:, 128:256]
    nc.tensor.transpose(oT_ps[:, 0:P], outacc[:P, :], ident[:P, :P])
    oT = work.tile([128, 128], F32, tag="oTsb")
    nc.scalar.activation(oT[:, 0:P], oT_ps[:, 0:P], AF.Copy)
    y_ps = pc[:, 256:320]
    nc.tensor.matmul(y_ps[:P], oT[:, 0:P], wo4[:, :])
    if has_bias:
        nc.vector.tensor_tensor(y_ps[:P], y_ps[:P], bias_o[:P, :], op=ALU.add)
    y_r = work.tile([128, OUT], F32, tag="y_r")
    nc.scalar.activation(y_r[:P], y_ps[:P], AF.Relu)
    nc.sync.dma_start(y[c0 : c0 + P, :], y_r[:P])


def _build(n_per_core, has_bias):
    key = (n_per_core, has_bias)
    if key in _BUILD_CACHE:
        return _BUILD_CACHE[key]

    nc = bacc.Bacc()
    agent = nc.dram_tensor("agent", [n_per_core, AGENT_SIZE], F32, kind="ExternalInput").ap()
    neigh = nc.dram_tensor(
        "neighbor", [n_per_core, NK * NEIGH_SIZE], F32, kind="ExternalInput"
    ).ap()
    mask = nc.dram_tensor("mask", [n_per_core, NK], I32, kind="ExternalInput").ap()
    wa = nc.dram_tensor("wa", [AGENT_SIZE, HM], F32, kind="ExternalInput").ap()
    wn = nc.dram_tensor("wn", [NEIGH_SIZE, HM], F32, kind="ExternalInput").ap()
    wh = nc.dram_tensor("wh", [NEIGH_SIZE, HM], F32, kind="ExternalInput").ap()
    wo4 = nc.dram_tensor("wo4", [HM, OUT], F32, kind="ExternalInput").ap()
    biases = nc.dram_tensor("biases", [1, 3 * HM + OUT], F32, kind="ExternalInput").ap()
    y = nc.dram_tensor("y", [n_per_core, OUT], F32, kind="ExternalOutput").ap()

    with ExitStack() as ctx:
        tc = ctx.enter_context(tile.TileContext(nc))
        const = ctx.enter_context(tc.tile_pool(name="const", bufs=1))
        inp = ctx.enter_context(tc.tile_pool(name="inp", bufs=3))
        xt_ps = ctx.enter_context(tc.tile_pool(name="xt_ps", bufs=2, space="PSUM"))
        xt_sb = ctx.enter_context(tc.tile_pool(name="xt_sb", bufs=2))
        proj_ps = ctx.enter_context(tc.tile_pool(name="proj_ps", bufs=1, space="PSUM"))
        work = ctx.enter_context(tc.tile_pool(name="work", bufs=2))
        out_ps = ctx.enter_context(tc.tile_pool(name="out_ps", bufs=2, space="PSUM"))

        ident = const.tile([128, 128], F32)
        masks.make_identity(nc, ident[:])
        wa_sb = const.tile([AGENT_SIZE, HM], F32)
        nc.sync.dma_start(wa_sb[:], wa[:, :])
        wn_sb = const.tile([NEIGH_SIZE, HM], F32)
        nc.sync.dma_start(wn_sb[:], wn[:, :])
        wh_sb = const.tile([NEIGH_SIZE, HM], F32)
        nc.sync.dma_start(wh_sb[:], wh[:, :])
        wo4_sb = const.tile([HM, OUT], F32)
        nc.sync.dma_start(wo4_sb[:], wo4[:, :])
        bias_abc = None
        bias_o = None
        if has_bias:
            bias_abc = const.tile([128, 3 * HM], F32)
            nc.sync.dma_start(
                bias_abc[:], biases[0:1, 0 : 3 * HM].broadcast_to([128, 3 * HM])
            )
            bias_o = const.tile([128, OUT], F32)
            nc.sync.dma_start(
                bias_o[:], biases[0:1, 3 * HM :].broadcast_to([128, OUT])
            )

        pools = (inp, xt_ps, xt_sb, proj_ps, work, out_ps, const)
        aps = (agent, neigh, mask, y, wa_sb, wn_sb, wh_sb, wo4_sb, bias_abc, bias_o, ident)

        n_full, rem = divmod(n_per_core, 128)
        for c in range(n_full):
            _emit_chunk(None, tc, pools, aps, c * 128, 128, has_bias)
        if rem:
            _emit_chunk(None, tc, pools, aps, n_full * 128, rem, has_bias)

    nc.compile()
    _BUILD_CACHE[key] = nc
    return nc


BF16 = mybir.dt.bfloat16


def _emit_block_v2(tc, pools, aps, b0, has_bias, stage=99):
    nc = tc.nc
    (inp, xtp, sbuf, psA, psT, psS) = pools
    (agent, neigh, mask, y, wst, hsel4, wo4, identb, maskc) = aps
    CH = 4

    ag = inp.tile([128, CH * 64], BF16, tag="ag")
    nb = inp.tile([128, CH * 256], BF16, tag="nb")
    mk = inp.tile([128, CH * NK], I32, tag="mk")
    nc.gpsimd.dma_start(
        ag[:, :], agent[b0 : b0 + 512, :].rearrange("(c p) f -> p c f", p=128)
    )
    nc.gpsimd.dma_start(
        nb[:, :], neigh[b0 : b0 + 512, :].rearrange("(c p) f -> p c f", p=128)
    )
    nc.sync.dma_start(
        mk[:, :], mask[b0 : b0 + 512, :].rearrange("(c p) k -> p c k", p=128)
    )

    xt_n01 = xtp.tile([128, 512], BF16, tag="xt01")
    xt_n23 = xtp.tile([128, 512], BF16, tag="xt23")
    xt_a = xtp.tile([128, 256], BF16, tag="xta")
    t_ps1 = psT.tile([128, 1024], BF16, tag="pt", name="t_ps1")
    for c in range(CH):
        nc.tensor.transpose(
            t_ps1[:, 128 * c : 128 * (c + 1)], nb[:, 256 * c : 256 * c + 128], identb[:, :]
        )
        nc.tensor.transpose(
            t_ps1[:, 512 + 128 * c : 640 + 128 * c],
            nb[:, 256 * c + 128 : 256 * c + 256],
            identb[:, :],
        )
    nc.vector.tensor_copy(xt_n01[:, :], t_ps1[:, 0:512])
    nc.vector.tensor_copy(xt_n23[:, :], t_ps1[:, 512:1024])
    t_ps2 = psT.tile([128, 256], BF16, tag="pt", name="t_ps2")
    nc.tensor.transpose(t_ps2[:, 0:128], ag[:, 0:128], identb[:, :])
    nc.tensor.transpose(t_ps2[:, 128:256], ag[:, 128:256], identb[:, :])
    nc.scalar.activation(xt_a[:, :], t_ps2[:, :], AF.Copy)

    def _bail(t):
        w = t.shape[-1]
        y_sb = sbuf.tile([128, 256], F32, tag="y_sb")
        if w < 256:
            nc.gpsimd.memset(y_sb[:, :], 0.0)
        nc.vector.tensor_copy(y_sb[:, 0:w], t)
        nc.sync.dma_start(
            y[b0 : b0 + 512, :].rearrange("(c p) f -> p c f", p=128), y_sb[:, :]
        )

    if stage <= 1:
        _bail(xt_n01.bitcast(F32)[:, 0:256])
        return

    a_ps0 = psS.tile([128, 256], F32, tag="ps", name="a_ps0")
    a_ps1 = psS.tile([128, 256], F32, tag="ps", name="a_ps1")
    nr_ps = [psA.tile([128, 512], F32, tag="pp", name=f"nr_ps{k}") for k in range(NK)]
    for u in range(2):
        nc.tensor.matmul(
            a_ps0[:, 128 * u : 128 * (u + 1)],
            wst[0:64, 0:128],
            xt_a[0:64, 128 * u : 128 * (u + 1)],
            tile_position=(0, 0),
        )
        nc.tensor.matmul(
            a_ps1[:, 128 * u : 128 * (u + 1)],
            wst[64:128, 0:128],
            xt_a[64:128, 128 * u : 128 * (u + 1)],
            tile_position=(64, 0),
        )

    for kp in range(2):
        xt = xt_n01 if kp == 0 else xt_n23
        nc.tensor.matmul(
            nr_ps[2 * kp][:, :], wst[0:64, 128:256], xt[0:64, :], tile_position=(0, 0)
        )
        nc.tensor.matmul(
            nr_ps[2 * kp + 1][:, :],
            wst[64:128, 128:256],
            xt[64:128, :],
            tile_position=(64, 0),
        )

    if stage <= 2:
        _bail(nr_ps[0][:, 0:256])
        return

    a_r = sbuf.tile([128, 512], BF16, tag="a_r")
    a_r_v = a_r.rearrange("p (u c f) -> p u c f", u=2, c=2)
    nc.scalar.activation(
        a_r_v.transpose([0, 2, 1, 3])[:, 0], a_ps0.rearrange("p (u f) -> p u f", u=2), AF.Relu
    )
    nc.scalar.activation(
        a_r_v.transpose([0, 2, 1, 3])[:, 1], a_ps1.rearrange("p (u f) -> p u f", u=2), AF.Relu
    )
    prods = []
    for k in range(NK):
        p_t = sbuf.tile([128, 512], BF16, tag=f"prod{k}")
        if k < 2:
            nc.vector.scalar_tensor_tensor(
                p_t[:, :], nr_ps[k][:, :], 0.0, a_r[:, :], op0=ALU.max, op1=ALU.mult
            )
        else:
            nr_r = sbuf.tile([128, 512], BF16, tag=f"nr_r{k}")
            nc.scalar.activation(nr_r[:, :], nr_ps[k][:, :], AF.Relu)
            nc.vector.tensor_tensor(p_t[:, :], nr_r[:, :], a_r[:, :], op=ALU.mult)
        prods.append(p_t)

    att_ps = psS.tile([128, 512], F32, tag="ps")
    for k in range(NK):
        nc.tensor.matmul(
            att_ps[32 * k : 32 * k + 32, :],
            hsel4[:, 32 * k : 32 * k + 32],
            prods[k][:, :],
            tile_position=(0, 32 * k),
        )
    if stage <= 3:
        _bail(att_ps[:, 0:256])
        return

    att_sb = sbuf.tile([128, 512], BF16, tag="attsb")
    nc.scalar.activation(att_sb[:, :], att_ps[:, :], AF.Copy)

    attT_oT = psS.tile([128, 1024], BF16, tag="ps")
    attT = attT_oT[:, 0:512]
    for c in range(CH):
        nc.tensor.transpose(
            attT[:, 128 * c : 128 * (c + 1)],
            att_sb[:, 128 * c : 128 * (c + 1)],
            identb[:, :],
        )

    mkp = sbuf.tile([128, CH * NK], F32, tag="mkp")
    nc.vector.tensor_scalar_mul(mkp[:, :], mk[:, :], -1.0e8)
    am = sbuf.tile([128, CH * 16], F32, tag="am")
    in_v = attT.rearrange("p (c r) -> p c r", c=CH)
    in_ckh = in_v.rearrange("p c (k r) -> p c k r", k=NK)[:, :, :, 0:4]
    mkp_ckh = mkp.rearrange("p (c k) -> p c k", c=CH).unsqueeze(3).broadcast_to(
        [128, CH, NK, NUM_HEAD]
    )
    am_ckh = am.rearrange("p (c h k) -> p c h k", c=CH, h=NUM_HEAD).transpose(
        [0, 1, 3, 2]
    )
    nc.vector.tensor_tensor(am_ckh, in_ckh, mkp_ckh, op=ALU.add)
    es = sbuf.tile([128, CH * 16], F32, tag="es")
    nc.scalar.activation(es[:, :], am[:, :], AF.Exp)
    ssum = sbuf.tile([128, CH * NUM_HEAD], F32, tag="ssum")
    nc.vector.tensor_reduce(
        ssum.rearrange("p (c h) -> p c h", c=CH),
        es.rearrange("p (c h k) -> p c h k", c=CH, h=NUM_HEAD),
        axis=AX.X,
        op=ALU.add,
    )
    rs = sbuf.tile([128, CH * NUM_HEAD], F32, tag="rs")
    nc.vector.tensor_scalar_max(ssum[:, :], ssum[:, :], 1.0e-30)
    nc.vector.reciprocal(rs[:, :], ssum[:, :])
    score = sbuf.tile([128, CH * 16], BF16, tag="score")
    nc.vector.tensor_tensor(
        score.rearrange("p (c h k) -> p c h k", c=CH, h=NUM_HEAD),
        es.rearrange("p (c h k) -> p c h k", c=CH, h=NUM_HEAD),
        rs.rearrange("p (c h) -> p c h", c=CH).unsqueeze(3).broadcast_to(
            [128, CH, NUM_HEAD, NK]
        ),
        op=ALU.mult,
    )

    if stage <= 4:
        _bail(es[:, :])
        return

    nh_ps = [psA.tile([128, 512], F32, tag="pp", name=f"nh_ps{k}") for k in range(NK)]
    for c in range(CH):
        for kp in range(2):
            xt = xt_n01 if kp == 0 else xt_n23
            nc.tensor.matmul(
                nh_ps[2 * kp][:, 128 * c : 128 * (c + 1)],
                xt[0:64, 128 * c : 128 * (c + 1)],
                wst[0:64, 256:384],
                tile_position=(0, 0),
            )
            nc.tensor.matmul(
                nh_ps[2 * kp + 1][:, 128 * c : 128 * (c + 1)],
                xt[64:128, 128 * c : 128 * (c + 1)],
                wst[64:128, 256:384],
                tile_position=(64, 0),
            )

    wks = []
    for k in range(NK):
        wk = sbuf.tile([128, 512], BF16, tag=f"wk{k}")
        sc_v = (
            score.rearrange("p (c h k) -> p c h k", c=CH, h=NUM_HEAD)[:, :, :, k]
            .unsqueeze(3)
            .broadcast_to([128, CH, NUM_HEAD, MID_SIZE])
        )
        nh_v = nh_ps[k].rearrange("p (c h m) -> p c h m", c=CH, h=NUM_HEAD)
        wk_v = wk.rearrange("p (c h m) -> p c h m", c=CH, h=NUM_HEAD)
        if k < 2:
            nc.vector.scalar_tensor_tensor(
                wk_v, nh_v, 0.0, sc_v, op0=ALU.max, op1=ALU.mult
            )
        else:
            nh_r = sbuf.tile([128, 512], BF16, tag=f"nh_r{k}")
            nc.scalar.activation(nh_r[:, :], nh_ps[k][:, :], AF.Relu)
            nc.gpsimd.tensor_tensor(
                wk_v, nh_r.rearrange("p (c h m) -> p c h m", c=CH, h=NUM_HEAD), sc_v,
                op=ALU.mult,
            )
        wks.append(wk)

    u01 = sbuf.tile([128, 512], BF16, tag="u01")
    u23 = sbuf.tile([128, 512], BF16, tag="u23")
    outacc = sbuf.tile([128, 512], BF16, tag="outacc")
    nc.gpsimd.tensor_tensor(u01[:, :], wks[0][:, :], wks[1][:, :], op=ALU.add)
    nc.gpsimd.tensor_tensor(u23[:, :], wks[2][:, :], wks[3][:, :], op=ALU.add)
    nc.gpsimd.tensor_tensor(outacc[:, :], u01[:, :], u23[:, :], op=ALU.add)

    if stage <= 5:
        _bail(outacc.bitcast(F32)[:, 0:256])
        return

    oT_ps = attT_oT[:, 512:1024]
    for c in range(CH):
        nc.tensor.transpose(
            oT_ps[:, 128 * c : 128 * (c + 1)],
            outacc[:, 128 * c : 128 * (c + 1)],
            identb[:, :],
        )
    oT = sbuf.tile([128, 512], BF16, tag="oTsb")
    nc.vector.tensor_copy(oT[:, :], oT_ps[:, :])
    y_ps = psS.tile([128, 256], F32, tag="ps")
    for c in range(CH):
        nc.tensor.matmul(
            y_ps[:, 64 * c : 64 * (c + 1)], oT[:, 128 * c : 128 * (c + 1)], wo4[:, :]
        )
    y_sb = sbuf.tile([128, 256], F32, tag="y_sb")
    nc.scalar.activation(y_sb[:, :], y_ps[:, :], AF.Relu)
    nc.sync.dma_start(
        y[b0 : b0 + 512, :].rearrange("(c p) f -> p c f", p=128), y_sb[:, :]
    )


def _build_v2(n_pad, stage=99):
    key = ("v2", n_pad, stage)
    if key in _BUILD_CACHE:
        return _BUILD_CACHE[key]
    assert n_pad % 512 == 0
    nc = bacc.Bacc()
    agent = nc.dram_tensor("agent", [n_pad, AGENT_SIZE], F32, kind="ExternalInput").ap()
    neigh = nc.dram_tensor(
        "neighbor", [n_pad, NK * NEIGH_SIZE], F32, kind="ExternalInput"
    ).ap()
    mask = nc.dram_tensor("mask", [n_pad, NK], I32, kind="ExternalInput").ap()
    wst_d = nc.dram_tensor("wst", [128, 384], BF16, kind="ExternalInput").ap()
    hsel_d = nc.dram_tensor("hsel", [128, 128], BF16, kind="ExternalInput").ap()
    wo4_d = nc.dram_tensor("wo4", [HM, OUT], BF16, kind="ExternalInput").ap()
    y = nc.dram_tensor("y", [n_pad, OUT], F32, kind="ExternalOutput").ap()

    with ExitStack() as ctx:
        tc = ctx.enter_context(tile.TileContext(nc))
        const = ctx.enter_context(tc.tile_pool(name="const", bufs=1))
        inp = ctx.enter_context(tc.tile_pool(name="inp", bufs=3))
        xtp = ctx.enter_context(tc.tile_pool(name="xtp", bufs=2))
        sbuf = ctx.enter_context(tc.tile_pool(name="sbuf", bufs=2))
        psA = ctx.enter_context(tc.tile_pool(name="psA", bufs=4, space="PSUM"))
        psT = ctx.enter_context(tc.tile_pool(name="psT", bufs=1, space="PSUM"))
        psS = ctx.enter_context(tc.tile_pool(name="psS", bufs=3, space="PSUM"))

        wst = const.tile([128, 384], BF16)
        nc.sync.dma_start(wst[:], wst_d[:, :])
        hsel4 = const.tile([128, 128], BF16)
        nc.sync.dma_start(hsel4[:], hsel_d[:, :])
        wo4 = const.tile([HM, OUT], BF16)
        nc.sync.dma_start(wo4[:], wo4_d[:, :])
        identb = const.tile([128, 128], BF16)
        masks.make_identity(nc, identb[:])

        pools = (inp, xtp, sbuf, psA, psT, psS)
        aps = (agent, neigh, mask, y, wst, hsel4, wo4, identb, None)
        for b in range(n_pad // 512):
            _emit_block_v2(tc, pools, aps, b * 512, False, stage=stage)

    nc.compile()
    _BUILD_CACHE[key] = nc
    return nc


def kernel(
    agent,
    neighbor,
    neighbor_mask,
    W_agent,
    b_agent,
    W_neigh,
    b_neigh,
    W_hid,
    b_hid,
    W_out,
    b_out,
    _trace=False,
):
    n = agent.shape[0]
    assert n % N_CORES == 0
    npc = n // N_CORES

    agent = np.ascontiguousarray(np.asarray(agent, dtype=np.float32))
    neighbor = np.ascontiguousarray(np.asarray(neighbor, dtype=np.float32)).reshape(n, NK * NEIGH_SIZE)
    neighbor_mask = np.ascontiguousarray(np.asarray(neighbor_mask, dtype=np.int32))

    biases = np.concatenate(
        [
            np.asarray(b_agent, np.float32).ravel(),
            np.asarray(b_neigh, np.float32).ravel(),
            np.asarray(b_hid, np.float32).ravel(),
            np.asarray(b_out, np.float32).ravel(),
        ]
    )[None, :]
    has_bias = bool(np.any(biases))
    use_v2 = (not has_bias) and os.environ.get("GAT_KERNEL_V1", "0") != "1"

    if use_v2:
        import ml_dtypes

        bf16 = ml_dtypes.bfloat16
        npad = ((npc + 511) // 512) * 512
        nc = _build_v2(npad)
        wa = np.asarray(W_agent, np.float32)
        wn = np.asarray(W_neigh, np.float32)
        wh = np.asarray(W_hid, np.float32)
        wst = np.concatenate(
            [
                np.concatenate([wa, wa], axis=0),
                np.concatenate([wn, wn], axis=0),
                np.concatenate([wh, wh], axis=0),
            ],
            axis=1,
        ).astype(bf16)
        hsel = np.zeros((128, 128), np.float32)
        for j in range(4):
            for h in range(4):
                hsel[h * 32 : (h + 1) * 32, 32 * j + h] = 1.0
        wmaps = {
            "wst": wst,
            "hsel": hsel.astype(bf16),
            "wo4": (np.asarray(W_out, np.float32) / 4.0).astype(bf16),
        }
        pad = npad - npc
        in_maps = []
        for i in range(N_CORES):
            sl = slice(i * npc, (i + 1) * npc)
            m = {
                "agent": np.pad(agent[sl], ((0, pad), (0, 0))),
                "neighbor": np.pad(neighbor[sl], ((0, pad), (0, 0))),
                "mask": np.pad(neighbor_mask[sl], ((0, pad), (0, 0))),
                **wmaps,
            }
            in_maps.append(m)
        res = run_bass_kernel_spmd(nc, in_maps, list(range(N_CORES)), trace=_trace)
        out = np.concatenate(
            [res.results[i]["y"][:npc] for i in range(N_CORES)], axis=0
        )
        if _trace:
            kernel._last_results = res
        return out

    nc = _build(npc, has_bias)

    wmaps = {
        "wa": np.asarray(W_agent, np.float32),
        "wn": np.asarray(W_neigh, np.float32),
        "wh": np.asarray(W_hid, np.float32),
        "wo4": np.asarray(W_out, np.float32) / 4.0,
        "biases": biases.astype(np.float32),
    }
    in_maps = []
    for i in range(N_CORES):
        sl = slice(i * npc, (i + 1) * npc)
        in_maps.append(
            {
                "agent": agent[sl],
                "neighbor": neighbor[sl],
                "mask": neighbor_mask[sl],
                **wmaps,
            }
        )

    res = run_bass_kernel_spmd(nc, in_maps, list(range(N_CORES)), trace=_trace)
    out = np.concatenate([res.results[i]["y"] for i in range(N_CORES)], axis=0)
    if _trace:
        kernel._last_results = res
    return out



# revision 6
# speedup vs baseline: 1.2232x; 1.2232x over previous
import os
import sys

if "/opt/trn_rl_repo" not in sys.path:
    sys.path.insert(0, "/opt/trn_rl_repo")

from contextlib import ExitStack

import numpy as np

import concourse.bass as bass
import concourse.tile as tile
from concourse import bacc, masks, mybir
from concourse.bass_utils import run_bass_kernel_spmd

F32 = mybir.dt.float32
I32 = mybir.dt.int32
AF = mybir.ActivationFunctionType
ALU = mybir.AluOpType
AX = mybir.AxisListType

N_CORES = 8
AGENT_SIZE = 64
NEIGH_SIZE = 64
NUM_HEAD = 4
MID_SIZE = 32
NK = 4
HM = NUM_HEAD * MID_SIZE
OUT = HM // 2

_BUILD_CACHE = {}


def _emit_chunk(ctx, tc, pools, aps, c0, P, has_bias):
    nc = tc.nc
    (inp, xt_ps, xt_sb, proj_ps, work, out_ps, const) = pools
    (agent, neigh, mask, y, wa, wn, wh, wo4, bias_abc, bias_o, ident) = aps

    ag = inp.tile([128, AGENT_SIZE], F32, tag="ag")
    nc.sync.dma_start(ag[:P], agent[c0 : c0 + P, :])
    nb = inp.tile([128, NK * NEIGH_SIZE], F32, tag="nb")
    nc.sync.dma_start(nb[:P], neigh[c0 : c0 + P, :])
    mk = inp.tile([128, NK], I32, tag="mk")
    nc.sync.dma_start(mk[:P], mask[c0 : c0 + P, :])

    t1 = xt_ps.tile([64, 512], F32, tag="t1")
    t2 = xt_ps.tile([64, 128], F32, tag="t2")
    nc.tensor.transpose(t1[:, 0:P], ag[:P, :], ident[:P, :P])
    for k in range(3):
        nc.tensor.transpose(
            t1[:, 128 * (k + 1) : 128 * (k + 1) + P],
            nb[:P, 64 * k : 64 * (k + 1)],
            ident[:P, :P],
        )
    nc.tensor.transpose(t2[:, 0:P], nb[:P, 64 * 3 : 64 * 4], ident[:P, :P])

    xt = xt_sb.tile([64, 640], F32, tag="xt")
    nc.scalar.activation(xt[:, 0:512], t1[:, :], AF.Copy)
    nc.scalar.activation(xt[:, 512:640], t2[:, :], AF.Copy)
    agT = lambda: xt[:, 0:P]
    nbT = lambda k: xt[:, 128 * (k + 1) : 128 * (k + 1) + P]

    pa = proj_ps.tile([128, 512], F32, tag="pa")
    pb = proj_ps.tile([128, 512], F32, tag="pb")
    pc = out_ps.tile([128, 320], F32, tag="pc")
    nc.tensor.matmul(pa[:P, 0:128], agT(), wa[:, :])
    for k in range(4):
        dst = pa[:P, 128 * (k + 1) : 128 * (k + 2)] if k < 3 else pb[:P, 0:128]
        nc.tensor.matmul(dst, nbT(k), wn[:, :])
    for k in range(4):
        dst = pb[:P, 128 * (k + 1) : 128 * (k + 2)] if k < 3 else pc[:P, 0:128]
        nc.tensor.matmul(dst, nbT(k), wh[:, :])

    a_psv = pa[:P, 0:128]
    nr_psv = [
        pa[:P, 128:256],
        pa[:P, 256:384],
        pa[:P, 384:512],
        pb[:P, 0:128],
    ]
    nh_psv = [
        pb[:P, 128:256],
        pb[:P, 256:384],
        pb[:P, 384:512],
        pc[:P, 0:128],
    ]

    ba = bias_abc
    a_r = work.tile([128, HM], F32, tag="a_r")
    nr_r = work.tile([128, 4 * HM], F32, tag="nr_r")
    nh_r = work.tile([128, 4 * HM], F32, tag="nh_r")
    if has_bias:
        nc.vector.tensor_tensor(a_psv, a_psv, ba[:P, 0:128], op=ALU.add)
        for k in range(4):
            nc.vector.tensor_tensor(nr_psv[k], nr_psv[k], ba[:P, 128:256], op=ALU.add)
            nc.vector.tensor_tensor(nh_psv[k], nh_psv[k], ba[:P, 256:384], op=ALU.add)
    nc.vector.tensor_scalar_max(a_r[:P], a_psv, 0.0)
    for k in range(4):
        dst_nr = nr_r[:P, 128 * k : 128 * (k + 1)]
        dst_nh = nh_r[:P, 128 * k : 128 * (k + 1)]
        if k < 3:
            nc.vector.tensor_scalar_max(dst_nr, nr_psv[k], 0.0)
            nc.scalar.activation(dst_nh, nh_psv[k], AF.Relu)
        else:
            nc.scalar.activation(dst_nr, nr_psv[k], AF.Relu)
            nc.scalar.activation(dst_nh, nh_psv[k], AF.Relu)

    att = work.tile([128, NUM_HEAD * NK], F32, tag="att")
    prod = work.tile([128, HM], F32, tag="prod")
    att_v = att[:P].rearrange("p (h k) -> p h k", k=NK)
    for k in range(4):
        nc.vector.tensor_tensor(
            prod[:P], a_r[:P], nr_r[:P, 128 * k : 128 * (k + 1)], op=ALU.mult
        )
        nc.vector.tensor_reduce(
            att_v[:, :, k],
            prod[:P].rearrange("p (h m) -> p h m", h=NUM_HEAD),
            axis=AX.X,
            op=ALU.add,
        )

    mkp = work.tile([128, NK], F32, tag="mkp")
    nc.vector.tensor_scalar_mul(mkp[:P], mk[:P], -1.0e8)
    attm = work.tile([128, NUM_HEAD * NK], F32, tag="attm")
    mkp_b = mkp[:P].unsqueeze(1).broadcast_to([P, NUM_HEAD, NK])
    nc.vector.tensor_tensor(
        attm[:P].rearrange("p (h k) -> p h k", k=NK), att_v, mkp_b, op=ALU.add
    )
    es = work.tile([128, NUM_HEAD * NK], F32, tag="es")
    nc.scalar.activation(es[:P], attm[:P], AF.Exp)
    s4 = work.tile([128, NUM_HEAD], F32, tag="s4")
    nc.vector.tensor_reduce(
        s4[:P], es[:P].rearrange("p (h k) -> p h k", k=NK), axis=AX.X, op=ALU.add
    )
    s4m = work.tile([128, NUM_HEAD], F32, tag="s4m")
    nc.vector.tensor_scalar_max(s4m[:P], s4[:P], 1.0e-30)
    r4 = work.tile([128, NUM_HEAD], F32, tag="r4")
    nc.vector.reciprocal(r4[:P], s4m[:P])
    score = work.tile([128, NUM_HEAD * NK], F32, tag="score")
    r4_b = r4[:P].unsqueeze(2).broadcast_to([P, NUM_HEAD, NK])
    nc.vector.tensor_tensor(
        score[:P].rearrange("p (h k) -> p h k", k=NK),
        es[:P].rearrange("p (h k) -> p h k", k=NK),
        r4_b,
        op=ALU.mult,
    )

    wk01 = work.tile([128, HM], F32, tag="wk01")
    wk23 = work.tile([128, HM], F32, tag="wk23")
    wkt = work.tile([128, HM], F32, tag="wkt")
    outacc = work.tile([128, HM], F32, tag="outacc")
    sc_v = score[:P].rearrange("p (h k) -> p h k", k=NK)

    def score_k(k):
        return sc_v[:, :, k].unsqueeze(2).broadcast_to([P, NUM_HEAD, MID_SIZE])

    def nh_k(k):
        return nh_r[:P, 128 * k : 128 * (k + 1)].rearrange(
            "p (h m) -> p h m", h=NUM_HEAD
        )

    ge = nc.gpsimd
    ge.tensor_tensor(
        wk01[:P].rearrange("p (h m) -> p h m", h=NUM_HEAD), score_k(0), nh_k(0), op=ALU.mult
    )
    ge.tensor_tensor(
        wkt[:P].rearrange("p (h m) -> p h m", h=NUM_HEAD), score_k(1), nh_k(1), op=ALU.mult
    )
    ge.tensor_tensor(wk01[:P], wk01[:P], wkt[:P], op=ALU.add)
    ge.tensor_tensor(
        wk23[:P].rearrange("p (h m) -> p h m", h=NUM_HEAD), score_k(2), nh_k(2), op=ALU.mult
    )
    ge.tensor_tensor(
        wkt[:P].rearrange("p (h m) -> p h m", h=NUM_HEAD), score_k(3), nh_k(3), op=ALU.mult
    )
    ge.tensor_tensor(wk23[:P], wk23[:P], wkt[:P], op=ALU.add)
    ge.tensor_tensor(outacc[:P], wk01[:P], wk23[:P], op=ALU.add)

    oT_ps = pc[:, 128:256]
    nc.tensor.transpose(oT_ps[:, 0:P], outacc[:P, :], ident[:P, :P])
    oT = work.tile([128, 128], F32, tag="oTsb")
    nc.scalar.activation(oT[:, 0:P], oT_ps[:, 0:P], AF.Copy)
    y_ps = pc[:, 256:320]
    nc.tensor.matmul(y_ps[:P], oT[:, 0:P], wo4[:, :])
    if has_bias:
        nc.vector.tensor_tensor(y_ps[:P], y_ps[:P], bias_o[:P, :], op=ALU.add)
    y_r = work.tile([128, OUT], F32, tag="y_r")
    nc.scalar.activation(y_r[:P], y_ps[:P], AF.Relu)
    nc.sync.dma_start(y[c0 : c0 + P, :], y_r[:P])


def _build(n_per_core, has_bias):
    key = (n_per_core, has_bias)
    if key in _BUILD_CACHE:
        return _BUILD_CACHE[key]

    nc = bacc.Bacc()
    agent = nc.dram_tensor("agent", [n_per_core, AGENT_SIZE], F32, kind="ExternalInput").ap()
    neigh = nc.dram_tensor(
        "neighbor", [n_per_core, NK * NEIGH_SIZE], F32, kind="ExternalInput"
    ).ap()
    mask = nc.dram_tensor("mask", [n_per_core, NK], I32, kind="ExternalInput").ap()
    wa = nc.dram_tensor("wa", [AGENT_SIZE, HM], F32, kind="ExternalInput").ap()
    wn = nc.dram_tensor("wn", [NEIGH_SIZE, HM], F32, kind="ExternalInput").ap()
    wh = nc.dram_tensor("wh", [NEIGH_SIZE, HM], F32, kind="ExternalInput").ap()
    wo4 = nc.dram_tensor("wo4", [HM, OUT], F32, kind="ExternalInput").ap()
    biases = nc.dram_tensor("biases", [1, 3 * HM + OUT], F32, kind="ExternalInput").ap()
    y = nc.dram_tensor("y", [n_per_core, OUT], F32, kind="ExternalOutput").ap()

    with ExitStack() as ctx:
        tc = ctx.enter_context(tile.TileContext(nc))
        const = ctx.enter_context(tc.tile_pool(name="const", bufs=1))
        inp = ctx.enter_context(tc.tile_pool(name="inp", bufs=3))
        xt_ps = ctx.enter_context(tc.tile_pool(name="xt_ps", bufs=2, space="PSUM"))
        xt_sb = ctx.enter_context(tc.tile_pool(name="xt_sb", bufs=2))
        proj_ps = ctx.enter_context(tc.tile_pool(name="proj_ps", bufs=1, space="PSUM"))
        work = ctx.enter_context(tc.tile_pool(name="work", bufs=2))
        out_ps = ctx.enter_context(tc.tile_pool(name="out_ps", bufs=2, space="PSUM"))

        ident = const.tile([128, 128], F32)
        masks.make_identity(nc, ident[:])
        wa_sb = const.tile([AGENT_SIZE, HM], F32)
        nc.sync.dma_start(wa_sb[:], wa[:, :])
        wn_sb = const.tile([NEIGH_SIZE, HM], F32)
        nc.sync.dma_start(wn_sb[:], wn[:, :])
        wh_sb = const.tile([NEIGH_SIZE, HM], F32)
        nc.sync.dma_start(wh_sb[:], wh[:, :])
        wo4_sb = const.tile([HM, OUT], F32)
        nc.sync.dma_start(wo4_sb[:], wo4[:, :])
        bias_abc = None
        bias_o = None
        if has_bias:
            bias_abc = const.tile([128, 3 * HM], F32)
            nc.sync.dma_start(
                bias_abc[:], biases[0:1, 0 : 3 * HM].broadcast_to([128, 3 * HM])
            )
            bias_o = const.tile([128, OUT], F32)
            nc.sync.dma_start(
                bias_o[:], biases[0:1, 3 * HM :].broadcast_to([128, OUT])
            )

        pools = (inp, xt_ps, xt_sb, proj_ps, work, out_ps, const)
        aps = (agent, neigh, mask, y, wa_sb, wn_sb, wh_sb, wo4_sb, bias_abc, bias_o, ident)

        n_full, rem = divmod(n_per_core, 128)
        for c in range(n_full):
            _emit_chunk(None, tc, pools, aps, c * 128, 128, has_bias)
        if rem:
            _emit_chunk(None, tc, pools, aps, n_full * 128, rem, has_bias)

    nc.compile()
    _BUILD_CACHE[key] = nc
    return nc


BF16 = mybir.dt.bfloat16


def _emit_block_v2(tc, pools, aps, b0, has_bias, stage=99):
    nc = tc.nc
    (inp, xtp, sbuf, psA, psT, psS) = pools
    (agent, neigh, mask, y, wst, hsel4, wo4, identb, maskc) = aps
    CH = 4

    ag = inp.tile([128, CH * 64], BF16, tag="ag")
    nb = inp.tile([128, CH * 256], BF16, tag="nb")
    mk = inp.tile([128, CH * NK], I32, tag="mk")
    nc.gpsimd.dma_start(
        ag[:, :], agent[b0 : b0 + 512, :].rearrange("(c p) f -> p c f", p=128)
    )
    nc.gpsimd.dma_start(
        nb[:, :], neigh[b0 : b0 + 512, :].rearrange("(c p) f -> p c f", p=128)
    )
    nc.sync.dma_start(
        mk[:, :], mask[b0 : b0 + 512, :].rearrange("(c p) k -> p c k", p=128)
    )

    xt_n01 = xtp.tile([128, 512], BF16, tag="xt01")
    xt_n23 = xtp.tile([128, 512], BF16, tag="xt23")
    xt_a = xtp.tile([128, 256], BF16, tag="xta")
    t_ps1 = psT.tile([128, 1024], BF16, tag="pt", name="t_ps1")
    for c in range(CH):
        nc.tensor.transpose(
            t_ps1[:, 128 * c : 128 * (c + 1)], nb[:, 256 * c : 256 * c + 128], identb[:, :]
        )
        nc.tensor.transpose(
            t_ps1[:, 512 + 128 * c : 640 + 128 * c],
            nb[:, 256 * c + 128 : 256 * c + 256],
            identb[:, :],
        )
    nc.vector.tensor_copy(xt_n01[:, :], t_ps1[:, 0:512])
    nc.vector.tensor_copy(xt_n23[:, :], t_ps1[:, 512:1024])
    t_ps2 = psT.tile([128, 256], BF16, tag="pt", name="t_ps2")
    nc.tensor.transpose(t_ps2[:, 0:128], ag[:, 0:128], identb[:, :])
    nc.tensor.transpose(t_ps2[:, 128:256], ag[:, 128:256], identb[:, :])
    nc.scalar.activation(xt_a[:, :], t_ps2[:, :], AF.Copy)

    def _bail(t):
        w = t.shape[-1]
        y_sb = sbuf.tile([128, 256], F32, tag="y_sb")
        if w < 256:
            nc.gpsimd.memset(y_sb[:, :], 0.0)
        nc.vector.tensor_copy(y_sb[:, 0:w], t)
        nc.sync.dma_start(
            y[b0 : b0 + 512, :].rearrange("(c p) f -> p c f", p=128), y_sb[:, :]
        )

    if stage <= 1:
        _bail(xt_n01.bitcast(F32)[:, 0:256])
        return

    a_ps0 = psS.tile([128, 256], F32, tag="ps", name="a_ps0")
    a_ps1 = psS.tile([128, 256], F32, tag="ps", name="a_ps1")
    nr_ps = [psA.tile([128, 512], F32, tag="pp", name=f"nr_ps{k}") for k in range(NK)]
    for u in range(2):
        nc.tensor.matmul(
            a_ps0[:, 128 * u : 128 * (u + 1)],
            wst[0:64, 0:128],
            xt_a[0:64, 128 * u : 128 * (u + 1)],
            tile_position=(0, 0),
        )
        nc.tensor.matmul(
            a_ps1[:, 128 * u : 128 * (u + 1)],
            wst[64:128, 0:128],
            xt_a[64:128, 128 * u : 128 * (u + 1)],
            tile_position=(64, 0),
        )

    for kp in range(2):
        xt = xt_n01 if kp == 0 else xt_n23
        nc.tensor.matmul(
            nr_ps[2 * kp][:, :], wst[0:64, 128:256], xt[0:64, :], tile_position=(0, 0)
        )
        nc.tensor.matmul(
            nr_ps[2 * kp + 1][:, :],
            wst[64:128, 128:256],
            xt[64:128, :],
            tile_position=(64, 0),
        )

    if stage <= 2:
        _bail(nr_ps[0][:, 0:256])
        return

    a_r = sbuf.tile([128, 512], BF16, tag="a_r")
    a_r_v = a_r.rearrange("p (u c f) -> p u c f", u=2, c=2)
    nc.scalar.activation(
        a_r_v.transpose([0, 2, 1, 3])[:, 0], a_ps0.rearrange("p (u f) -> p u f", u=2), AF.Relu
    )
    nc.scalar.activation(
        a_r_v.transpose([0, 2, 1, 3])[:, 1], a_ps1.rearrange("p (u f) -> p u f", u=2), AF.Relu
    )
    prods = []
    for k in range(NK):
        p_t = sbuf.tile([128, 512], BF16, tag=f"prod{k}")
        if k < 2:
            nc.vector.scalar_tensor_tensor(
                p_t[:, :], nr_ps[k][:, :], 0.0, a_r[:, :], op0=ALU.max, op1=ALU.mult
            )
        else:
            nr_r = sbuf.tile([128, 512], BF16, tag=f"nr_r{k}")
            nc.scalar.activation(nr_r[:, :], nr_ps[k][:, :], AF.Relu)
            nc.vector.tensor_tensor(p_t[:, :], nr_r[:, :], a_r[:, :], op=ALU.mult)
        prods.append(p_t)

    att_ps = psS.tile([128, 512], F32, tag="ps")
    for k in range(NK):
        nc.tensor.matmul(
            att_ps[32 * k : 32 * k + 32, :],
            hsel4[:, 32 * k : 32 * k + 32],
            prods[k][:, :],
            tile_position=(0, 32 * k),
        )
    if stage <= 3:
        _bail(att_ps[:, 0:256])
        return

    att_sb = sbuf.tile([128, 512], BF16, tag="attsb")
    nc.scalar.activation(att_sb[:, :], att_ps[:, :], AF.Copy)

    attT_oT = psS.tile([128, 1024], BF16, tag="ps")
    attT = attT_oT[:, 0:512]
    for c in range(CH):
        nc.tensor.transpose(
            attT[:, 128 * c : 128 * (c + 1)],
            att_sb[:, 128 * c : 128 * (c + 1)],
            identb[:, :],
        )

    mkp = sbuf.tile([128, CH * NK], F32, tag="mkp")
    nc.vector.tensor_scalar_mul(mkp[:, :], mk[:, :], -1.0e8)
    am = sbuf.tile([128, CH * 16], F32, tag="am")
    in_v = attT.rearrange("p (c r) -> p c r", c=CH)
    in_ckh = in_v.rearrange("p c (k r) -> p c k r", k=NK)[:, :, :, 0:4]
    mkp_ckh = mkp.rearrange("p (c k) -> p c k", c=CH).unsqueeze(3).broadcast_to(
        [128, CH, NK, NUM_HEAD]
    )
    am_ckh = am.rearrange("p (c h k) -> p c h k", c=CH, h=NUM_HEAD).transpose(
        [0, 1, 3, 2]
    )
    nc.vector.tensor_tensor(am_ckh, in_ckh, mkp_ckh, op=ALU.add)
    es = sbuf.tile([128, CH * 16], F32, tag="es")
    nc.scalar.activation(es[:, :], am[:, :], AF.Exp)
    ssum = sbuf.tile([128, CH * NUM_HEAD], F32, tag="ssum")
    nc.vector.tensor_reduce(
        ssum.rearrange("p (c h) -> p c h", c=CH),
        es.rearrange("p (c h k) -> p c h k", c=CH, h=NUM_HEAD),
        axis=AX.X,
        op=ALU.add,
    )
    rs = sbuf.tile([128, CH * NUM_HEAD], F32, tag="rs")
    nc.vector.tensor_scalar_max(ssum[:, :], ssum[:, :], 1.0e-30)
    nc.vector.reciprocal(rs[:, :], ssum[:, :])
    score = sbuf.tile([128, CH * 16], BF16, tag="score")
    nc.vector.tensor_tensor(
        score.rearrange("p (c h k) -> p c h k", c=CH, h=NUM_HEAD),
        es.rearrange("p (c h k) -> p c h k", c=CH, h=NUM_HEAD),
        rs.rearrange("p (c h) -> p c h", c=CH).unsqueeze(3).broadcast_to(
            [128, CH, NUM_HEAD, NK]
        ),
        op=ALU.mult,
    )

    if stage <= 4:
        _bail(es[:, :])
        return

    nh_ps = [psA.tile([128, 512], F32, tag="pp", name=f"nh_ps{k}") for k in range(NK)]
    for c in range(CH):
        for kp in range(2):
            xt = xt_n01 if kp == 0 else xt_n23
            nc.tensor.matmul(
                nh_ps[2 * kp][:, 128 * c : 128 * (c + 1)],
                xt[0:64, 128 * c : 128 * (c + 1)],
                wst[0:64, 256:384],
                tile_position=(0, 0),
            )
            nc.tensor.matmul(
                nh_ps[2 * kp + 1][:, 128 * c : 128 * (c + 1)],
                xt[64:128, 128 * c : 128 * (c + 1)],
                wst[64:128, 256:384],
                tile_position=(64, 0),
            )

    wks = []
    for k in range(NK):
        wk = sbuf.tile([128, 512], BF16, tag=f"wk{k}")
        sc_v = (
            score.rearrange("p (c h k) -> p c h k", c=CH, h=NUM_HEAD)[:, :, :, k]
            .unsqueeze(3)
            .broadcast_to([128, CH, NUM_HEAD, MID_SIZE])
        )
        nh_v = nh_ps[k].rearrange("p (c h m) -> p c h m", c=CH, h=NUM_HEAD)
        wk_v = wk.rearrange("p (c h m) -> p c h m", c=CH, h=NUM_HEAD)
        if k < 2:
            nc.vector.scalar_tensor_tensor(
                wk_v, nh_v, 0.0, sc_v, op0=ALU.max, op1=ALU.mult
            )
        else:
            nh_r = sbuf.tile([128, 512], BF16, tag=f"nh_r{k}")
            nc.scalar.activation(nh_r[:, :], nh_ps[k][:, :], AF.Relu)
            nc.gpsimd.tensor_tensor(
                wk_v, nh_r.rearrange("p (c h m) -> p c h m", c=CH, h=NUM_HEAD), sc_v,
                op=ALU.mult,
            )
        wks.append(wk)

    u01 = sbuf.tile([128, 512], BF16, tag="u01")
    u23 = sbuf.tile([128, 512], BF16, tag="u23")
    outacc = sbuf.tile([128, 512], BF16, tag="outacc")
    nc.gpsimd.tensor_tensor(u01[:, :], wks[0][:, :], wks[1][:, :], op=ALU.add)
    nc.gpsimd.tensor_tensor(u23[:, :], wks[2][:, :], wks[3][:, :], op=ALU.add)
    nc.gpsimd.tensor_tensor(outacc[:, :], u01[:, :], u23[:, :], op=ALU.add)

    if stage <= 5:
        _bail(outacc.bitcast(F32)[:, 0:256])
        return

    oT_ps = attT_oT[:, 512:1024]
    for c in range(CH):
        nc.tensor.transpose(
            oT_ps[:, 128 * c : 128 * (c + 1)],
            outacc[:, 128 * c : 128 * (c + 1)],
            identb[:, :],
        )
    oT = sbuf.tile([128, 512], BF16, tag="oTsb")
    nc.vector.tensor_copy(oT[:, :], oT_ps[:, :])
    y_ps = psS.tile([128, 256], F32, tag="ps")
    for c in range(CH):
        nc.tensor.matmul(
            y_ps[:, 64 * c : 64 * (c + 1)], oT[:, 128 * c : 128 * (c + 1)], wo4[:, :]
        )
    y_sb = sbuf.tile([128, 256], F32, tag="y_sb")
    nc.scalar.activation(y_sb[:, :], y_ps[:, :], AF.Relu)
    nc.sync.dma_start(
        y[b0 : b0 + 512, :].rearrange("(c p) f -> p c f", p=128), y_sb[:, :]
    )


def _emit_block_v4(tc, pools, aps, b0):
    nc = tc.nc
    (inp, sbuf, psP, psQ) = pools
    (xt, mask, y, wstk, hsel4, wo4, identb) = aps
    CH = 4
    NA = 512

    xtAB = inp.tile([128, 1024], BF16, tag="xtAB")
    nc.sync.dma_start(
        xtAB.rearrange("p (s a) -> p s a", s=2),
        xt[0:256, b0 : b0 + NA].rearrange("(s p) a -> p s a", p=128),
    )
    xtC = inp.tile([64, 512], BF16, tag="xtC")
    nc.sync.dma_start(xtC[:, :], xt[256:320, b0 : b0 + NA])
    mk = inp.tile([128, CH * NK], I32, tag="mk")
    nc.sync.dma_start(
        mk[:, :], mask[b0 : b0 + NA, :].rearrange("(c p) k -> p c k", p=128)
    )

    ag = xtAB[0:64, 0:512]
    nb = [xtAB[64:128, 0:512], xtAB[0:64, 512:1024], xtAB[64:128, 512:1024], xtC[0:64, :]]
    w_a_top = wstk[0:64, 0:128]
    w_n_bot = wstk[64:128, 0:128]
    w_n_top = wstk[0:64, 128:256]
    w_h_bot = wstk[64:128, 256:384]
    w_h_top = wstk[0:64, 256:384]

    a_ps = psQ.tile([128, 512], F32, tag="q", name="a_ps")
    pr01 = psP.tile([128, 1024], F32, tag="p", name="pr01")
    pr23 = psP.tile([128, 1024], F32, tag="p", name="pr23")
    nc.tensor.matmul(a_ps[:, :], w_a_top, ag, tile_position=(0, 0))
    nc.tensor.matmul(pr01[:, 0:512], w_n_bot, nb[0], tile_position=(64, 0))
    nc.tensor.matmul(pr01[:, 512:1024], w_n_top, nb[1], tile_position=(0, 0))
    nc.tensor.matmul(pr23[:, 0:512], w_n_bot, nb[2], tile_position=(64, 0))
    nc.tensor.matmul(pr23[:, 512:1024], w_n_top, nb[3], tile_position=(0, 0))

    a_r = sbuf.tile([128, 512], BF16, tag="a_r")
    nc.scalar.activation(a_r[:, :], a_ps[:, :], AF.Relu)
    a_r_b = a_r.rearrange("p (u a) -> p u a", u=1).broadcast_to([128, 2, 512])
    prods01 = sbuf.tile([128, 1024], BF16, tag="prods01")
    prods23 = sbuf.tile([128, 1024], BF16, tag="prods23")
    nc.vector.scalar_tensor_tensor(
        prods01.rearrange("p (u a) -> p u a", u=2), pr01.rearrange("p (u a) -> p u a", u=2),
        0.0, a_r_b, op0=ALU.max, op1=ALU.mult,
    )
    nr23_r = sbuf.tile([128, 1024], BF16, tag="nr23r")
    nc.scalar.activation(nr23_r[:, :], pr23[:, :], AF.Relu)
    nc.vector.tensor_tensor(
        prods23.rearrange("p (u a) -> p u a", u=2),
        nr23_r.rearrange("p (u a) -> p u a", u=2), a_r_b, op=ALU.mult,
    )

    att_ps = psQ.tile([128, 512], F32, tag="q", name="att_ps")
    prods_k = [prods01[:, 0:512], prods01[:, 512:1024], prods23[:, 0:512], prods23[:, 512:1024]]
    for k in range(NK):
        nc.tensor.matmul(
            att_ps[32 * k : 32 * k + 32, :], hsel4[:, 32 * k : 32 * k + 32],
            prods_k[k], tile_position=(0, 32 * k),
        )
    att_sb = sbuf.tile([128, 512], BF16, tag="attsb")
    nc.scalar.activation(att_sb[:, :], att_ps[:, :], AF.Copy)

    attT = psQ.tile([128, 512], BF16, tag="q", name="attT")
    for c in range(CH):
        nc.tensor.transpose(
            attT[:, 128 * c : 128 * (c + 1)], att_sb[:, 128 * c : 128 * (c + 1)],
            identb[:, :],
        )

    ph01 = psP.tile([128, 1024], F32, tag="p", name="ph01")
    ph23 = psP.tile([128, 1024], F32, tag="p", name="ph23")
    for c in range(CH):
        nc.tensor.matmul(
            ph01[:, 128 * c : 128 * (c + 1)],
            xtAB[64:128, 128 * c : 128 * (c + 1)], w_h_bot, tile_position=(64, 0),
        )
        nc.tensor.matmul(
            ph01[:, 512 + 128 * c : 640 + 128 * c],
            xtAB[0:64, 512 + 128 * c : 640 + 128 * c], w_h_top, tile_position=(0, 0),
        )
    for c in range(CH):
        nc.tensor.matmul(
            ph23[:, 128 * c : 128 * (c + 1)],
            xtAB[64:128, 512 + 128 * c : 640 + 128 * c], w_h_bot, tile_position=(64, 0),
        )
        nc.tensor.matmul(
            ph23[:, 512 + 128 * c : 640 + 128 * c],
            xtC[0:64, 128 * c : 128 * (c + 1)], w_h_top, tile_position=(0, 0),
        )

    mkp = sbuf.tile([128, CH * NK], F32, tag="mkp")
    nc.vector.tensor_scalar_mul(mkp[:, :], mk[:, :], -1.0e8)
    am = sbuf.tile([128, CH * 16], F32, tag="am")
    in_v = attT.rearrange("p (c r) -> p c r", c=CH)
    in_ckh = in_v.rearrange("p c (k r) -> p c k r", k=NK)[:, :, :, 0:4]
    mkp_ckh = mkp.rearrange("p (c k) -> p c k", c=CH).unsqueeze(3).broadcast_to(
        [128, CH, NK, NUM_HEAD]
    )
    am_ckh = am.rearrange("p (c h k) -> p c h k", c=CH, h=NUM_HEAD).transpose([0, 1, 3, 2])
    nc.vector.tensor_tensor(am_ckh, in_ckh, mkp_ckh, op=ALU.add)
    es = sbuf.tile([128, CH * 16], F32, tag="es")
    nc.scalar.activation(es[:, :], am[:, :], AF.Exp)
    ssum = sbuf.tile([128, CH * NUM_HEAD], F32, tag="ssum")
    nc.vector.tensor_reduce(
        ssum.rearrange("p (c h) -> p c h", c=CH),
        es.rearrange("p (c h k) -> p c h k", c=CH, h=NUM_HEAD),
        axis=AX.X, op=ALU.add,
    )
    rs = sbuf.tile([128, CH * NUM_HEAD], F32, tag="rs")
    nc.vector.tensor_scalar_max(ssum[:, :], ssum[:, :], 1.0e-30)
    nc.vector.reciprocal(rs[:, :], ssum[:, :])
    score = sbuf.tile([128, CH * 16], BF16, tag="score")
    nc.vector.tensor_tensor(
        score.rearrange("p (c h k) -> p c h k", c=CH, h=NUM_HEAD),
        es.rearrange("p (c h k) -> p c h k", c=CH, h=NUM_HEAD),
        rs.rearrange("p (c h) -> p c h", c=CH).unsqueeze(3).broadcast_to(
            [128, CH, NUM_HEAD, NK]
        ),
        op=ALU.mult,
    )
    score_v = score.rearrange("p (c h k) -> p c h k", c=CH, h=NUM_HEAD)

    def score_b3(k, dims=3):
        s = score_v[:, :, :, k].rearrange("p c h -> p (c h)")
        return s.unsqueeze(2).broadcast_to([128, CH * NUM_HEAD, MID_SIZE])

    def wk_v3(t, half):
        return t[:, 512 * half : 512 * (half + 1)].rearrange(
            "p (q m) -> p q m", m=MID_SIZE
        )

    ph01_r = sbuf.tile([128, 1024], BF16, tag="ph01r")
    nc.scalar.activation(ph01_r[:, :], ph01[:, :], AF.Relu)
    wk01 = sbuf.tile([128, 1024], BF16, tag="wk01")
    wk23 = sbuf.tile([128, 1024], BF16, tag="wk23")
    nc.gpsimd.tensor_tensor(wk_v3(wk01, 0), wk_v3(ph01_r, 0), score_b3(0), op=ALU.mult)
    nc.gpsimd.tensor_tensor(wk_v3(wk01, 1), wk_v3(ph01_r, 1), score_b3(1), op=ALU.mult)
    nc.vector.scalar_tensor_tensor(
        wk_v3(wk23, 0), wk_v3(ph23, 0), 0.0, score_b3(2), op0=ALU.max, op1=ALU.mult
    )
    nc.vector.scalar_tensor_tensor(
        wk_v3(wk23, 1), wk_v3(ph23, 1), 0.0, score_b3(3), op0=ALU.max, op1=ALU.mult
    )

    u01 = sbuf.tile([128, 512], BF16, tag="u01")
    u23 = sbuf.tile([128, 512], BF16, tag="u23")
    outacc = sbuf.tile([128, 512], BF16, tag="outacc")
    nc.vector.tensor_tensor(u01[:, :], wk01[:, 0:512], wk01[:, 512:1024], op=ALU.add)
    nc.gpsimd.tensor_tensor(u23[:, :], wk23[:, 0:512], wk23[:, 512:1024], op=ALU.add)
    nc.vector.tensor_tensor(outacc[:, :], u01[:, :], u23[:, :], op=ALU.add)

    oT_ps = psQ.tile([128, 512], BF16, tag="q", name="oT_ps")
    for c in range(CH):
        nc.tensor.transpose(
            oT_ps[:, 128 * c : 128 * (c + 1)], outacc[:, 128 * c : 128 * (c + 1)],
            identb[:, :],
        )
    oT = sbuf.tile([128, 512], BF16, tag="oTsb")
    nc.scalar.activation(oT[:, :], oT_ps[:, :], AF.Copy)
    y_ps = psQ.tile([128, 256], F32, tag="q", name="y_ps")
    for c in range(CH):
        nc.tensor.matmul(
            y_ps[:, 64 * c : 64 * (c + 1)], oT[:, 128 * c : 128 * (c + 1)], wo4[:, :]
        )
    y_sb = sbuf.tile([128, 256], F32, tag="y_sb")
    nc.scalar.activation(y_sb[:, :], y_ps[:, :], AF.Relu)
    nc.sync.dma_start(
        y[b0 : b0 + NA, :].rearrange("(c p) f -> p c f", p=128), y_sb[:, :]
    )


def _build_v4(n_pad):
    key = ("v4", n_pad)
    if key in _BUILD_CACHE:
        return _BUILD_CACHE[key]
    assert n_pad % 512 == 0
    nc = bacc.Bacc()
    xt = nc.dram_tensor("xt", [320, n_pad], BF16, kind="ExternalInput").ap()
    mask = nc.dram_tensor("mask", [n_pad, NK], I32, kind="ExternalInput").ap()
    wstk_d = nc.dram_tensor("wstk", [128, 384], BF16, kind="ExternalInput").ap()
    hsel_d = nc.dram_tensor("hsel", [128, 128], BF16, kind="ExternalInput").ap()
    wo4_d = nc.dram_tensor("wo4", [HM, OUT], BF16, kind="ExternalInput").ap()
    y = nc.dram_tensor("y", [n_pad, OUT], F32, kind="ExternalOutput").ap()

    with ExitStack() as ctx:
        tc = ctx.enter_context(tile.TileContext(nc))
        const = ctx.enter_context(tc.tile_pool(name="const", bufs=1))
        inp = ctx.enter_context(tc.tile_pool(name="inp", bufs=3))
        sbuf = ctx.enter_context(tc.tile_pool(name="sbuf", bufs=2))
        psP = ctx.enter_context(tc.tile_pool(name="psP", bufs=3, space="PSUM"))
        psQ = ctx.enter_context(tc.tile_pool(name="psQ", bufs=2, space="PSUM"))

        wstk = const.tile([128, 384], BF16)
        nc.sync.dma_start(wstk[:], wstk_d[:, :])
        hsel4 = const.tile([128, 128], BF16)
        nc.sync.dma_start(hsel4[:], hsel_d[:, :])
        wo4 = const.tile([HM, OUT], BF16)
        nc.sync.dma_start(wo4[:], wo4_d[:, :])
        identb = const.tile([128, 128], BF16)
        masks.make_identity(nc, identb[:])

        pools = (inp, sbuf, psP, psQ)
        aps = (xt, mask, y, wstk, hsel4, wo4, identb)
        for b in range(n_pad // 512):
            _emit_block_v4(tc, pools, aps, b * 512)

    nc.compile()
    _BUILD_CACHE[key] = nc
    return nc


def _build_v2(n_pad, stage=99):
    key = ("v2", n_pad, stage)
    if key in _BUILD_CACHE:
        return _BUILD_CACHE[key]
    assert n_pad % 512 == 0
    nc = bacc.Bacc()
    agent = nc.dram_tensor("agent", [n_pad, AGENT_SIZE], F32, kind="ExternalInput").ap()
    neigh = nc.dram_tensor(
        "neighbor", [n_pad, NK * NEIGH_SIZE], F32, kind="ExternalInput"
    ).ap()
    mask = nc.dram_tensor("mask", [n_pad, NK], I32, kind="ExternalInput").ap()
    wst_d = nc.dram_tensor("wst", [128, 384], BF16, kind="ExternalInput").ap()
    hsel_d = nc.dram_tensor("hsel", [128, 128], BF16, kind="ExternalInput").ap()
    wo4_d = nc.dram_tensor("wo4", [HM, OUT], BF16, kind="ExternalInput").ap()
    y = nc.dram_tensor("y", [n_pad, OUT], F32, kind="ExternalOutput").ap()

    with ExitStack() as ctx:
        tc = ctx.enter_context(tile.TileContext(nc))
        const = ctx.enter_context(tc.tile_pool(name="const", bufs=1))
        inp = ctx.enter_context(tc.tile_pool(name="inp", bufs=3))
        xtp = ctx.enter_context(tc.tile_pool(name="xtp", bufs=2))
        sbuf = ctx.enter_context(tc.tile_pool(name="sbuf", bufs=2))
        psA = ctx.enter_context(tc.tile_pool(name="psA", bufs=4, space="PSUM"))
        psT = ctx.enter_context(tc.tile_pool(name="psT", bufs=1, space="PSUM"))
        psS = ctx.enter_context(tc.tile_pool(name="psS", bufs=3, space="PSUM"))

        wst = const.tile([128, 384], BF16)
        nc.sync.dma_start(wst[:], wst_d[:, :])
        hsel4 = const.tile([128, 128], BF16)
        nc.sync.dma_start(hsel4[:], hsel_d[:, :])
        wo4 = const.tile([HM, OUT], BF16)
        nc.sync.dma_start(wo4[:], wo4_d[:, :])
        identb = const.tile([128, 128], BF16)
        masks.make_identity(nc, identb[:])

        pools = (inp, xtp, sbuf, psA, psT, psS)
        aps = (agent, neigh, mask, y, wst, hsel4, wo4, identb, None)
        for b in range(n_pad // 512):
            _emit_block_v2(tc, pools, aps, b * 512, False, stage=stage)

    nc.compile()
    _BUILD_CACHE[key] = nc
    return nc


def kernel(
    agent,
    neighbor,
    neighbor_mask,
    W_agent,
    b_agent,
    W_neigh,
    b_neigh,
    W_hid,
    b_hid,
    W_out,
    b_out,
    _trace=False,
):
    n = agent.shape[0]
    assert n % N_CORES == 0
    npc = n // N_CORES

    agent = np.ascontiguousarray(np.asarray(agent, dtype=np.float32))
    neighbor = np.ascontiguousarray(np.asarray(neighbor, dtype=np.float32)).reshape(n, NK * NEIGH_SIZE)
    neighbor_mask = np.ascontiguousarray(np.asarray(neighbor_mask, dtype=np.int32))

    biases = np.concatenate(
        [
            np.asarray(b_agent, np.float32).ravel(),
            np.asarray(b_neigh, np.float32).ravel(),
            np.asarray(b_hid, np.float32).ravel(),
            np.asarray(b_out, np.float32).ravel(),
        ]
    )[None, :]
    has_bias = bool(np.any(biases))
    kver = os.environ.get("GAT_KERNEL_VER", "4")
    use_v4 = (not has_bias) and kver == "4"
    use_v2 = (not has_bias) and not use_v4 and kver != "1"

    if use_v4:
        import ml_dtypes

        bf16 = ml_dtypes.bfloat16
        npad = ((npc + 511) // 512) * 512
        nc = _build_v4(npad)
        wa = np.asarray(W_agent, np.float32)
        wn = np.asarray(W_neigh, np.float32)
        wh = np.asarray(W_hid, np.float32)
        wstk = np.concatenate(
            [
                np.concatenate([wa, wn], axis=0),
                np.concatenate([wn, wh], axis=0),
                np.concatenate([wh, wh], axis=0),
            ],
            axis=1,
        ).astype(bf16)
        hsel = np.zeros((128, 128), np.float32)
        for j in range(4):
            for h in range(4):
                hsel[h * 32 : (h + 1) * 32, 32 * j + h] = 1.0
        wmaps = {
            "wstk": wstk,
            "hsel": hsel.astype(bf16),
            "wo4": (np.asarray(W_out, np.float32) / 4.0).astype(bf16),
        }
        Xbf = np.concatenate([agent, neighbor], axis=1).astype(bf16)
        pad = npad - npc
        in_maps = []
        for i in range(N_CORES):
            sl = slice(i * npc, (i + 1) * npc)
            xt_c = np.zeros((320, npad), bf16)
            xt_c[:, :npc] = Xbf[sl].T
            m = {
                "xt": xt_c,
                "mask": np.pad(neighbor_mask[sl], ((0, pad), (0, 0))),
                **wmaps,
            }
            in_maps.append(m)
        res = run_bass_kernel_spmd(nc, in_maps, list(range(N_CORES)), trace=_trace)
        out = np.concatenate(
            [res.results[i]["y"][:npc] for i in range(N_CORES)], axis=0
        )
        if _trace:
            kernel._last_results = res
        return out

    if use_v2:
        import ml_dtypes

        bf16 = ml_dtypes.bfloat16
        npad = ((npc + 511) // 512) * 512
        nc = _build_v2(npad)
        wa = np.asarray(W_agent, np.float32)
        wn = np.asarray(W_neigh, np.float32)
        wh = np.asarray(W_hid, np.float32)
        wst = np.concatenate(
            [
                np.concatenate([wa, wa], axis=0),
                np.concatenate([wn, wn], axis=0),
                np.concatenate([wh, wh], axis=0),
            ],
            axis=1,
        ).astype(bf16)
        hsel = np.zeros((128, 128), np.float32)
        for j in range(4):
            for h in range(4):
                hsel[h * 32 : (h + 1) * 32, 32 * j + h] = 1.0
        wmaps = {
            "wst": wst,
            "hsel": hsel.astype(bf16),
            "wo4": (np.asarray(W_out, np.float32) / 4.0).astype(bf16),
        }
        pad = npad - npc
        in_maps = []
        for i in range(N_CORES):
            sl = slice(i * npc, (i + 1) * npc)
            m = {
                "agent": np.pad(agent[sl], ((0, pad), (0, 0))),
                "neighbor": np.pad(neighbor[sl], ((0, pad), (0, 0))),
                "mask": np.pad(neighbor_mask[sl], ((0, pad), (0, 0))),
                **wmaps,
            }
            in_maps.append(m)
        res = run_bass_kernel_spmd(nc, in_maps, list(range(N_CORES)), trace=_trace)
        out = np.concatenate(
            [res.results[i]["y"][:npc] for i in range(N_CORES)], axis=0
        )
        if _trace:
            kernel._last_results = res
        return out

    nc = _build(npc, has_bias)

    wmaps = {
        "wa": np.asarray(W_agent, np.float32),
        "wn": np.asarray(W_neigh, np.float32),
        "wh": np.asarray(W_hid, np.float32),
        "wo4": np.asarray(W_out, np.float32) / 4.0,
        "biases": biases.astype(np.float32),
    }
    in_maps = []
    for i in range(N_CORES):
        sl = slice(i * npc, (i + 1) * npc)
        in_maps.append(
            {
                "agent": agent[sl],
                "neighbor": neighbor[sl],
                "mask": neighbor_mask[sl],
                **wmaps,
            }
        )

    res = run_bass_kernel_spmd(nc, in_maps, list(range(N_CORES)), trace=_trace)
    out = np.concatenate([res.results[i]["y"] for i in range(N_CORES)], axis=0)
    if _trace:
        kernel._last_results = res
    return out



# revision 10
# speedup vs baseline: 1.4663x; 1.1987x over previous
import os
import sys

if "/opt/trn_rl_repo" not in sys.path:
    sys.path.insert(0, "/opt/trn_rl_repo")

from contextlib import ExitStack

import numpy as np

import concourse.bass as bass
import concourse.tile as tile
from concourse import bacc, masks, mybir
from concourse.bass_utils import run_bass_kernel_spmd

F32 = mybir.dt.float32
I32 = mybir.dt.int32
AF = mybir.ActivationFunctionType
ALU = mybir.AluOpType
AX = mybir.AxisListType

N_CORES = 8
AGENT_SIZE = 64
NEIGH_SIZE = 64
NUM_HEAD = 4
MID_SIZE = 32
NK = 4
HM = NUM_HEAD * MID_SIZE
OUT = HM // 2

_BUILD_CACHE = {}


def _emit_chunk(ctx, tc, pools, aps, c0, P, has_bias):
    nc = tc.nc
    (inp, xt_ps, xt_sb, proj_ps, work, out_ps, const) = pools
    (agent, neigh, mask, y, wa, wn, wh, wo4, bias_abc, bias_o, ident) = aps

    ag = inp.tile([128, AGENT_SIZE], F32, tag="ag")
    nc.sync.dma_start(ag[:P], agent[c0 : c0 + P, :])
    nb = inp.tile([128, NK * NEIGH_SIZE], F32, tag="nb")
    nc.sync.dma_start(nb[:P], neigh[c0 : c0 + P, :])
    mk = inp.tile([128, NK], I32, tag="mk")
    nc.sync.dma_start(mk[:P], mask[c0 : c0 + P, :])

    t1 = xt_ps.tile([64, 512], F32, tag="t1")
    t2 = xt_ps.tile([64, 128], F32, tag="t2")
    nc.tensor.transpose(t1[:, 0:P], ag[:P, :], ident[:P, :P])
    for k in range(3):
        nc.tensor.transpose(
            t1[:, 128 * (k + 1) : 128 * (k + 1) + P],
            nb[:P, 64 * k : 64 * (k + 1)],
            ident[:P, :P],
        )
    nc.tensor.transpose(t2[:, 0:P], nb[:P, 64 * 3 : 64 * 4], ident[:P, :P])

    xt = xt_sb.tile([64, 640], F32, tag="xt")
    nc.scalar.activation(xt[:, 0:512], t1[:, :], AF.Copy)
    nc.scalar.activation(xt[:, 512:640], t2[:, :], AF.Copy)
    agT = lambda: xt[:, 0:P]
    nbT = lambda k: xt[:, 128 * (k + 1) : 128 * (k + 1) + P]

    pa = proj_ps.tile([128, 512], F32, tag="pa")
    pb = proj_ps.tile([128, 512], F32, tag="pb")
    pc = out_ps.tile([128, 320], F32, tag="pc")
    nc.tensor.matmul(pa[:P, 0:128], agT(), wa[:, :])
    for k in range(4):
        dst = pa[:P, 128 * (k + 1) : 128 * (k + 2)] if k < 3 else pb[:P, 0:128]
        nc.tensor.matmul(dst, nbT(k), wn[:, :])
    for k in range(4):
        dst = pb[:P, 128 * (k + 1) : 128 * (k + 2)] if k < 3 else pc[:P, 0:128]
        nc.tensor.matmul(dst, nbT(k), wh[:, :])

    a_psv = pa[:P, 0:128]
    nr_psv = [
        pa[:P, 128:256],
        pa[:P, 256:384],
        pa[:P, 384:512],
        pb[:P, 0:128],
    ]
    nh_psv = [
        pb[:P, 128:256],
        pb[:P, 256:384],
        pb[:P, 384:512],
        pc[:P, 0:128],
    ]

    ba = bias_abc
    a_r = work.tile([128, HM], F32, tag="a_r")
    nr_r = work.tile([128, 4 * HM], F32, tag="nr_r")
    nh_r = work.tile([128, 4 * HM], F32, tag="nh_r")
    if has_bias:
        nc.vector.tensor_tensor(a_psv, a_psv, ba[:P, 0:128], op=ALU.add)
        for k in range(4):
            nc.vector.tensor_tensor(nr_psv[k], nr_psv[k], ba[:P, 128:256], op=ALU.add)
            nc.vector.tensor_tensor(nh_psv[k], nh_psv[k], ba[:P, 256:384], op=ALU.add)
    nc.vector.tensor_scalar_max(a_r[:P], a_psv, 0.0)
    for k in range(4):
        dst_nr = nr_r[:P, 128 * k : 128 * (k + 1)]
        dst_nh = nh_r[:P, 128 * k : 128 * (k + 1)]
        if k < 3:
            nc.vector.tensor_scalar_max(dst_nr, nr_psv[k], 0.0)
            nc.scalar.activation(dst_nh, nh_psv[k], AF.Relu)
        else:
            nc.scalar.activation(dst_nr, nr_psv[k], AF.Relu)
            nc.scalar.activation(dst_nh, nh_psv[k], AF.Relu)

    att = work.tile([128, NUM_HEAD * NK], F32, tag="att")
    prod = work.tile([128, HM], F32, tag="prod")
    att_v = att[:P].rearrange("p (h k) -> p h k", k=NK)
    for k in range(4):
        nc.vector.tensor_tensor(
            prod[:P], a_r[:P], nr_r[:P, 128 * k : 128 * (k + 1)], op=ALU.mult
        )
        nc.vector.tensor_reduce(
            att_v[:, :, k],
            prod[:P].rearrange("p (h m) -> p h m", h=NUM_HEAD),
            axis=AX.X,
            op=ALU.add,
        )

    mkp = work.tile([128, NK], F32, tag="mkp")
    nc.vector.tensor_scalar_mul(mkp[:P], mk[:P], -1.0e8)
    attm = work.tile([128, NUM_HEAD * NK], F32, tag="attm")
    mkp_b = mkp[:P].unsqueeze(1).broadcast_to([P, NUM_HEAD, NK])
    nc.vector.tensor_tensor(
        attm[:P].rearrange("p (h k) -> p h k", k=NK), att_v, mkp_b, op=ALU.add
    )
    es = work.tile([128, NUM_HEAD * NK], F32, tag="es")
    nc.scalar.activation(es[:P], attm[:P], AF.Exp)
    s4 = work.tile([128, NUM_HEAD], F32, tag="s4")
    nc.vector.tensor_reduce(
        s4[:P], es[:P].rearrange("p (h k) -> p h k", k=NK), axis=AX.X, op=ALU.add
    )
    s4m = work.tile([128, NUM_HEAD], F32, tag="s4m")
    nc.vector.tensor_scalar_max(s4m[:P], s4[:P], 1.0e-30)
    r4 = work.tile([128, NUM_HEAD], F32, tag="r4")
    nc.vector.reciprocal(r4[:P], s4m[:P])
    score = work.tile([128, NUM_HEAD * NK], F32, tag="score")
    r4_b = r4[:P].unsqueeze(2).broadcast_to([P, NUM_HEAD, NK])
    nc.vector.tensor_tensor(
        score[:P].rearrange("p (h k) -> p h k", k=NK),
        es[:P].rearrange("p (h k) -> p h k", k=NK),
        r4_b,
        op=ALU.mult,
    )

    wk01 = work.tile([128, HM], F32, tag="wk01")
    wk23 = work.tile([128, HM], F32, tag="wk23")
    wkt = work.tile([128, HM], F32, tag="wkt")
    outacc = work.tile([128, HM], F32, tag="outacc")
    sc_v = score[:P].rearrange("p (h k) -> p h k", k=NK)

    def score_k(k):
        return sc_v[:, :, k].unsqueeze(2).broadcast_to([P, NUM_HEAD, MID_SIZE])

    def nh_k(k):
        return nh_r[:P, 128 * k : 128 * (k + 1)].rearrange(
            "p (h m) -> p h m", h=NUM_HEAD
        )

    ge = nc.gpsimd
    ge.tensor_tensor(
        wk01[:P].rearrange("p (h m) -> p h m", h=NUM_HEAD), score_k(0), nh_k(0), op=ALU.mult
    )
    ge.tensor_tensor(
        wkt[:P].rearrange("p (h m) -> p h m", h=NUM_HEAD), score_k(1), nh_k(1), op=ALU.mult
    )
    ge.tensor_tensor(wk01[:P], wk01[:P], wkt[:P], op=ALU.add)
    ge.tensor_tensor(
        wk23[:P].rearrange("p (h m) -> p h m", h=NUM_HEAD), score_k(2), nh_k(2), op=ALU.mult
    )
    ge.tensor_tensor(
        wkt[:P].rearrange("p (h m) -> p h m", h=NUM_HEAD), score_k(3), nh_k(3), op=ALU.mult
    )
    ge.tensor_tensor(wk23[:P], wk23[:P], wkt[:P], op=ALU.add)
    ge.tensor_tensor(outacc[:P], wk01[:P], wk23[:P], op=ALU.add)

    oT_ps = pc[:, 128:256]
    nc.tensor.transpose(oT_ps[:, 0:P], outacc[:P, :], ident[:P, :P])
    oT = work.tile([128, 128], F32, tag="oTsb")
    nc.scalar.activation(oT[:, 0:P], oT_ps[:, 0:P], AF.Copy)
    y_ps = pc[:, 256:320]
    nc.tensor.matmul(y_ps[:P], oT[:, 0:P], wo4[:, :])
    if has_bias:
        nc.vector.tensor_tensor(y_ps[:P], y_ps[:P], bias_o[:P, :], op=ALU.add)
    y_r = work.tile([128, OUT], F32, tag="y_r")
    nc.scalar.activation(y_r[:P], y_ps[:P], AF.Relu)
    nc.sync.dma_start(y[c0 : c0 + P, :], y_r[:P])


def _build(n_per_core, has_bias):
    key = (n_per_core, has_bias)
    if key in _BUILD_CACHE:
        return _BUILD_CACHE[key]

    nc = bacc.Bacc()
    agent = nc.dram_tensor("agent", [n_per_core, AGENT_SIZE], F32, kind="ExternalInput").ap()
    neigh = nc.dram_tensor(
        "neighbor", [n_per_core, NK * NEIGH_SIZE], F32, kind="ExternalInput"
    ).ap()
    mask = nc.dram_tensor("mask", [n_per_core, NK], I32, kind="ExternalInput").ap()
    wa = nc.dram_tensor("wa", [AGENT_SIZE, HM], F32, kind="ExternalInput").ap()
    wn = nc.dram_tensor("wn", [NEIGH_SIZE, HM], F32, kind="ExternalInput").ap()
    wh = nc.dram_tensor("wh", [NEIGH_SIZE, HM], F32, kind="ExternalInput").ap()
    wo4 = nc.dram_tensor("wo4", [HM, OUT], F32, kind="ExternalInput").ap()
    biases = nc.dram_tensor("biases", [1, 3 * HM + OUT], F32, kind="ExternalInput").ap()
    y = nc.dram_tensor("y", [n_per_core, OUT], F32, kind="ExternalOutput").ap()

    with ExitStack() as ctx:
        tc = ctx.enter_context(tile.TileContext(nc))
        const = ctx.enter_context(tc.tile_pool(name="const", bufs=1))
        inp = ctx.enter_context(tc.tile_pool(name="inp", bufs=3))
        xt_ps = ctx.enter_context(tc.tile_pool(name="xt_ps", bufs=2, space="PSUM"))
        xt_sb = ctx.enter_context(tc.tile_pool(name="xt_sb", bufs=2))
        proj_ps = ctx.enter_context(tc.tile_pool(name="proj_ps", bufs=1, space="PSUM"))
        work = ctx.enter_context(tc.tile_pool(name="work", bufs=2))
        out_ps = ctx.enter_context(tc.tile_pool(name="out_ps", bufs=2, space="PSUM"))

        ident = const.tile([128, 128], F32)
        masks.make_identity(nc, ident[:])
        wa_sb = const.tile([AGENT_SIZE, HM], F32)
        nc.sync.dma_start(wa_sb[:], wa[:, :])
        wn_sb = const.tile([NEIGH_SIZE, HM], F32)
        nc.sync.dma_start(wn_sb[:], wn[:, :])
        wh_sb = const.tile([NEIGH_SIZE, HM], F32)
        nc.sync.dma_start(wh_sb[:], wh[:, :])
        wo4_sb = const.tile([HM, OUT], F32)
        nc.sync.dma_start(wo4_sb[:], wo4[:, :])
        bias_abc = None
        bias_o = None
        if has_bias:
            bias_abc = const.tile([128, 3 * HM], F32)
            nc.sync.dma_start(
                bias_abc[:], biases[0:1, 0 : 3 * HM].broadcast_to([128, 3 * HM])
            )
            bias_o = const.tile([128, OUT], F32)
            nc.sync.dma_start(
                bias_o[:], biases[0:1, 3 * HM :].broadcast_to([128, OUT])
            )

        pools = (inp, xt_ps, xt_sb, proj_ps, work, out_ps, const)
        aps = (agent, neigh, mask, y, wa_sb, wn_sb, wh_sb, wo4_sb, bias_abc, bias_o, ident)

        n_full, rem = divmod(n_per_core, 128)
        for c in range(n_full):
            _emit_chunk(None, tc, pools, aps, c * 128, 128, has_bias)
        if rem:
            _emit_chunk(None, tc, pools, aps, n_full * 128, rem, has_bias)

    nc.compile()
    _BUILD_CACHE[key] = nc
    return nc


BF16 = mybir.dt.bfloat16


def _emit_block_v2(tc, pools, aps, b0, has_bias, stage=99):
    nc = tc.nc
    (inp, xtp, sbuf, psA, psT, psS) = pools
    (agent, neigh, mask, y, wst, hsel4, wo4, identb, maskc) = aps
    CH = 4

    ag = inp.tile([128, CH * 64], BF16, tag="ag")
    nb = inp.tile([128, CH * 256], BF16, tag="nb")
    mk = inp.tile([128, CH * NK], I32, tag="mk")
    nc.gpsimd.dma_start(
        ag[:, :], agent[b0 : b0 + 512, :].rearrange("(c p) f -> p c f", p=128)
    )
    nc.gpsimd.dma_start(
        nb[:, :], neigh[b0 : b0 + 512, :].rearrange("(c p) f -> p c f", p=128)
    )
    nc.sync.dma_start(
        mk[:, :], mask[b0 : b0 + 512, :].rearrange("(c p) k -> p c k", p=128)
    )

    xt_n01 = xtp.tile([128, 512], BF16, tag="xt01")
    xt_n23 = xtp.tile([128, 512], BF16, tag="xt23")
    xt_a = xtp.tile([128, 256], BF16, tag="xta")
    t_ps1 = psT.tile([128, 1024], BF16, tag="pt", name="t_ps1")
    for c in range(CH):
        nc.tensor.transpose(
            t_ps1[:, 128 * c : 128 * (c + 1)], nb[:, 256 * c : 256 * c + 128], identb[:, :]
        )
        nc.tensor.transpose(
            t_ps1[:, 512 + 128 * c : 640 + 128 * c],
            nb[:, 256 * c + 128 : 256 * c + 256],
            identb[:, :],
        )
    nc.vector.tensor_copy(xt_n01[:, :], t_ps1[:, 0:512])
    nc.vector.tensor_copy(xt_n23[:, :], t_ps1[:, 512:1024])
    t_ps2 = psT.tile([128, 256], BF16, tag="pt", name="t_ps2")
    nc.tensor.transpose(t_ps2[:, 0:128], ag[:, 0:128], identb[:, :])
    nc.tensor.transpose(t_ps2[:, 128:256], ag[:, 128:256], identb[:, :])
    nc.scalar.activation(xt_a[:, :], t_ps2[:, :], AF.Copy)

    def _bail(t):
        w = t.shape[-1]
        y_sb = sbuf.tile([128, 256], F32, tag="y_sb")
        if w < 256:
            nc.gpsimd.memset(y_sb[:, :], 0.0)
        nc.vector.tensor_copy(y_sb[:, 0:w], t)
        nc.sync.dma_start(
            y[b0 : b0 + 512, :].rearrange("(c p) f -> p c f", p=128), y_sb[:, :]
        )

    if stage <= 1:
        _bail(xt_n01.bitcast(F32)[:, 0:256])
        return

    a_ps0 = psS.tile([128, 256], F32, tag="ps", name="a_ps0")
    a_ps1 = psS.tile([128, 256], F32, tag="ps", name="a_ps1")
    nr_ps = [psA.tile([128, 512], F32, tag="pp", name=f"nr_ps{k}") for k in range(NK)]
    for u in range(2):
        nc.tensor.matmul(
            a_ps0[:, 128 * u : 128 * (u + 1)],
            wst[0:64, 0:128],
            xt_a[0:64, 128 * u : 128 * (u + 1)],
            tile_position=(0, 0),
        )
        nc.tensor.matmul(
            a_ps1[:, 128 * u : 128 * (u + 1)],
            wst[64:128, 0:128],
            xt_a[64:128, 128 * u : 128 * (u + 1)],
            tile_position=(64, 0),
        )

    for kp in range(2):
        xt = xt_n01 if kp == 0 else xt_n23
        nc.tensor.matmul(
            nr_ps[2 * kp][:, :], wst[0:64, 128:256], xt[0:64, :], tile_position=(0, 0)
        )
        nc.tensor.matmul(
            nr_ps[2 * kp + 1][:, :],
            wst[64:128, 128:256],
            xt[64:128, :],
            tile_position=(64, 0),
        )

    if stage <= 2:
        _bail(nr_ps[0][:, 0:256])
        return

    a_r = sbuf.tile([128, 512], BF16, tag="a_r")
    a_r_v = a_r.rearrange("p (u c f) -> p u c f", u=2, c=2)
    nc.scalar.activation(
        a_r_v.transpose([0, 2, 1, 3])[:, 0], a_ps0.rearrange("p (u f) -> p u f", u=2), AF.Relu
    )
    nc.scalar.activation(
        a_r_v.transpose([0, 2, 1, 3])[:, 1], a_ps1.rearrange("p (u f) -> p u f", u=2), AF.Relu
    )
    prods = []
    for k in range(NK):
        p_t = sbuf.tile([128, 512], BF16, tag=f"prod{k}")
        if k < 2:
            nc.vector.scalar_tensor_tensor(
                p_t[:, :], nr_ps[k][:, :], 0.0, a_r[:, :], op0=ALU.max, op1=ALU.mult
            )
        else:
            nr_r = sbuf.tile([128, 512], BF16, tag=f"nr_r{k}", name=f"nr_r{k}")
            nc.scalar.activation(nr_r[:, :], nr_ps[k][:, :], AF.Relu)
            nc.vector.tensor_tensor(p_t[:, :], nr_r[:, :], a_r[:, :], op=ALU.mult)
        prods.append(p_t)

    att_ps = psS.tile([128, 512], F32, tag="ps")
    for k in range(NK):
        nc.tensor.matmul(
            att_ps[32 * k : 32 * k + 32, :],
            hsel4[:, 32 * k : 32 * k + 32],
            prods[k][:, :],
            tile_position=(0, 32 * k),
        )
    if stage <= 3:
        _bail(att_ps[:, 0:256])
        return

    att_sb = sbuf.tile([128, 512], BF16, tag="attsb")
    nc.scalar.activation(att_sb[:, :], att_ps[:, :], AF.Copy)

    attT_oT = psS.tile([128, 1024], BF16, tag="ps")
    attT = attT_oT[:, 0:512]
    for c in range(CH):
        nc.tensor.transpose(
            attT[:, 128 * c : 128 * (c + 1)],
            att_sb[:, 128 * c : 128 * (c + 1)],
            identb[:, :],
        )

    mkp = sbuf.tile([128, CH * NK], F32, tag="mkp")
    nc.vector.tensor_scalar_mul(mkp[:, :], mk[:, :], -1.0e8)
    am = sbuf.tile([128, CH * 16], F32, tag="am")
    in_v = attT.rearrange("p (c r) -> p c r", c=CH)
    in_ckh = in_v.rearrange("p c (k r) -> p c k r", k=NK)[:, :, :, 0:4]
    mkp_ckh = mkp.rearrange("p (c k) -> p c k", c=CH).unsqueeze(3).broadcast_to(
        [128, CH, NK, NUM_HEAD]
    )
    am_ckh = am.rearrange("p (c h k) -> p c h k", c=CH, h=NUM_HEAD).transpose(
        [0, 1, 3, 2]
    )
    nc.vector.tensor_tensor(am_ckh, in_ckh, mkp_ckh, op=ALU.add)
    es = sbuf.tile([128, CH * 16], F32, tag="es")
    nc.scalar.activation(es[:, :], am[:, :], AF.Exp)
    ssum = sbuf.tile([128, CH * NUM_HEAD], F32, tag="ssum")
    nc.vector.tensor_reduce(
        ssum.rearrange("p (c h) -> p c h", c=CH),
        es.rearrange("p (c h k) -> p c h k", c=CH, h=NUM_HEAD),
        axis=AX.X,
        op=ALU.add,
    )
    rs = sbuf.tile([128, CH * NUM_HEAD], F32, tag="rs")
    nc.vector.tensor_scalar_max(ssum[:, :], ssum[:, :], 1.0e-30)
    nc.vector.reciprocal(rs[:, :], ssum[:, :])
    score = sbuf.tile([128, CH * 16], BF16, tag="score")
    nc.vector.tensor_tensor(
        score.rearrange("p (c h k) -> p c h k", c=CH, h=NUM_HEAD),
        es.rearrange("p (c h k) -> p c h k", c=CH, h=NUM_HEAD),
        rs.rearrange("p (c h) -> p c h", c=CH).unsqueeze(3).broadcast_to(
            [128, CH, NUM_HEAD, NK]
        ),
        op=ALU.mult,
    )

    if stage <= 4:
        _bail(es[:, :])
        return

    nh_ps = [psA.tile([128, 512], F32, tag="pp", name=f"nh_ps{k}") for k in range(NK)]
    for c in range(CH):
        for kp in range(2):
            xt = xt_n01 if kp == 0 else xt_n23
            nc.tensor.matmul(
                nh_ps[2 * kp][:, 128 * c : 128 * (c + 1)],
                xt[0:64, 128 * c : 128 * (c + 1)],
                wst[0:64, 256:384],
                tile_position=(0, 0),
            )
            nc.tensor.matmul(
                nh_ps[2 * kp + 1][:, 128 * c : 128 * (c + 1)],
                xt[64:128, 128 * c : 128 * (c + 1)],
                wst[64:128, 256:384],
                tile_position=(64, 0),
            )

    wks = []
    for k in range(NK):
        wk = sbuf.tile([128, 512], BF16, tag=f"wk{k}")
        sc_v = (
            score.rearrange("p (c h k) -> p c h k", c=CH, h=NUM_HEAD)[:, :, :, k]
            .unsqueeze(3)
            .broadcast_to([128, CH, NUM_HEAD, MID_SIZE])
        )
        nh_v = nh_ps[k].rearrange("p (c h m) -> p c h m", c=CH, h=NUM_HEAD)
        wk_v = wk.rearrange("p (c h m) -> p c h m", c=CH, h=NUM_HEAD)
        if k < 2:
            nc.vector.scalar_tensor_tensor(
                wk_v, nh_v, 0.0, sc_v, op0=ALU.max, op1=ALU.mult
            )
        else:
            nh_r = sbuf.tile([128, 512], BF16, tag=f"nh_r{k}")
            nc.scalar.activation(nh_r[:, :], nh_ps[k][:, :], AF.Relu)
            nc.gpsimd.tensor_tensor(
                wk_v, nh_r.rearrange("p (c h m) -> p c h m", c=CH, h=NUM_HEAD), sc_v,
                op=ALU.mult,
            )
        wks.append(wk)

    u01 = sbuf.tile([128, 512], BF16, tag="u01")
    u23 = sbuf.tile([128, 512], BF16, tag="u23")
    outacc = sbuf.tile([128, 512], BF16, tag="outacc")
    nc.gpsimd.tensor_tensor(u01[:, :], wks[0][:, :], wks[1][:, :], op=ALU.add)
    nc.gpsimd.tensor_tensor(u23[:, :], wks[2][:, :], wks[3][:, :], op=ALU.add)
    nc.gpsimd.tensor_tensor(outacc[:, :], u01[:, :], u23[:, :], op=ALU.add)

    if stage <= 5:
        _bail(outacc.bitcast(F32)[:, 0:256])
        return

    oT_ps = attT_oT[:, 512:1024]
    for c in range(CH):
        nc.tensor.transpose(
            oT_ps[:, 128 * c : 128 * (c + 1)],
            outacc[:, 128 * c : 128 * (c + 1)],
            identb[:, :],
        )
    oT = sbuf.tile([128, 512], BF16, tag="oTsb")
    nc.vector.tensor_copy(oT[:, :], oT_ps[:, :])
    y_ps = psS.tile([128, 256], F32, tag="ps")
    for c in range(CH):
        nc.tensor.matmul(
            y_ps[:, 64 * c : 64 * (c + 1)], oT[:, 128 * c : 128 * (c + 1)], wo4[:, :]
        )
    y_sb = sbuf.tile([128, 256], F32, tag="y_sb")
    nc.scalar.activation(y_sb[:, :], y_ps[:, :], AF.Relu)
    nc.sync.dma_start(
        y[b0 : b0 + 512, :].rearrange("(c p) f -> p c f", p=128), y_sb[:, :]
    )


def _emit_block_v4(tc, pools, aps, b0):
    nc = tc.nc
    (inp, sbuf, psPR, psPH, psA, psTX) = pools
    (xt, mask, y, wstk, hsel4, wo4, identb) = aps
    CH = 4
    NA = 512

    xtAB = inp.tile([128, 1024], BF16, tag="xtAB")
    nc.sync.dma_start(
        xtAB.rearrange("p (s a) -> p s a", s=2),
        xt[0:256, b0 : b0 + NA].rearrange("(s p) a -> p s a", p=128),
    )
    xtC = inp.tile([64, 512], BF16, tag="xtC")
    nc.sync.dma_start(xtC[:, :], xt[256:320, b0 : b0 + NA])
    mk = inp.tile([128, CH * NK], I32, tag="mk")
    nc.sync.dma_start(
        mk[:, :], mask[b0 : b0 + NA, :].rearrange("(c p) k -> p c k", p=128)
    )

    ag = xtAB[0:64, 0:512]
    nb = [xtAB[64:128, 0:512], xtAB[0:64, 512:1024], xtAB[64:128, 512:1024], xtC[0:64, :]]
    w_a_top = wstk[0:64, 0:128]
    w_n_bot = wstk[64:128, 0:128]
    w_n_top = wstk[0:64, 128:256]
    w_h_bot = wstk[64:128, 256:384]
    w_h_top = wstk[0:64, 256:384]

    a_ps = psA.tile([128, 512], F32, tag="a", name="a_ps")
    pr = [psPR.tile([128, 512], F32, tag="pr", name=f"pr{k}") for k in range(NK)]
    nc.tensor.matmul(a_ps[:, :], w_a_top, ag, tile_position=(0, 0))
    nc.tensor.matmul(pr[0][:, :], w_n_bot, nb[0], tile_position=(64, 0))
    nc.tensor.matmul(pr[1][:, :], w_n_top, nb[1], tile_position=(0, 0))
    nc.tensor.matmul(pr[2][:, :], w_n_bot, nb[2], tile_position=(64, 0))
    nc.tensor.matmul(pr[3][:, :], w_n_top, nb[3], tile_position=(0, 0))

    a_r = sbuf.tile([128, 512], BF16, tag="a_r")
    nc.scalar.activation(a_r[:, :], a_ps[:, :], AF.Relu)
    prods = [sbuf.tile([128, 512], BF16, tag=f"prods{k}", name=f"prods{k}") for k in range(NK)]
    for k in (0, 1):
        nc.vector.scalar_tensor_tensor(
            prods[k][:, :], pr[k][:, :], 0.0, a_r[:, :], op0=ALU.max, op1=ALU.mult
        )
    for k in (2, 3):
        nr_r = sbuf.tile([128, 512], BF16, tag=f"nr_r{k}", name=f"nr_r{k}")
        nc.scalar.activation(nr_r[:, :], pr[k][:, :], AF.Relu)
        nc.vector.tensor_tensor(prods[k][:, :], nr_r[:, :], a_r[:, :], op=ALU.mult)

    att_ps = psTX.tile([128, 512], F32, tag="tx", name="att_ps")
    for k in range(NK):
        nc.tensor.matmul(
            att_ps[32 * k : 32 * k + 32, :], hsel4[:, 32 * k : 32 * k + 32],
            prods[k][:, :], tile_position=(0, 32 * k),
        )
    att_sb = sbuf.tile([128, 512], BF16, tag="attsb")
    nc.scalar.activation(att_sb[:, :], att_ps[:, :], AF.Copy)

    x_ps = psTX.tile([128, 1024], BF16, tag="tx", name="x_ps")
    attT = x_ps[:, 0:512]
    oT_ps = x_ps[:, 512:1024]
    for c in range(CH):
        nc.tensor.transpose(
            attT[:, 128 * c : 128 * (c + 1)], att_sb[:, 128 * c : 128 * (c + 1)],
            identb[:, :],
        )

    mkp = sbuf.tile([128, CH * NK], F32, tag="mkp")
    nc.vector.tensor_scalar_mul(mkp[:, :], mk[:, :], -1.0e8)
    am = sbuf.tile([128, CH * 16], F32, tag="am")
    in_v = attT.rearrange("p (c r) -> p c r", c=CH)
    in_ckh = in_v.rearrange("p c (k r) -> p c k r", k=NK)[:, :, :, 0:4]
    mkp_ckh = mkp.rearrange("p (c k) -> p c k", c=CH).unsqueeze(3).broadcast_to(
        [128, CH, NK, NUM_HEAD]
    )
    am_ckh = am.rearrange("p (c h k) -> p c h k", c=CH, h=NUM_HEAD).transpose([0, 1, 3, 2])
    nc.vector.tensor_tensor(am_ckh, in_ckh, mkp_ckh, op=ALU.add)
    es = sbuf.tile([128, CH * 16], F32, tag="es")
    nc.scalar.activation(es[:, :], am[:, :], AF.Exp)
    ssum = sbuf.tile([128, CH * NUM_HEAD], F32, tag="ssum")
    nc.vector.tensor_reduce(
        ssum.rearrange("p (c h) -> p c h", c=CH),
        es.rearrange("p (c h k) -> p c h k", c=CH, h=NUM_HEAD),
        axis=AX.X, op=ALU.add,
    )
    rs = sbuf.tile([128, CH * NUM_HEAD], F32, tag="rs")
    nc.vector.tensor_scalar_max(ssum[:, :], ssum[:, :], 1.0e-30)
    nc.vector.reciprocal(rs[:, :], ssum[:, :])
    score = sbuf.tile([128, CH * 16], BF16, tag="score")
    nc.vector.tensor_tensor(
        score.rearrange("p (c h k) -> p c h k", c=CH, h=NUM_HEAD),
        es.rearrange("p (c h k) -> p c h k", c=CH, h=NUM_HEAD),
        rs.rearrange("p (c h) -> p c h", c=CH).unsqueeze(3).broadcast_to(
            [128, CH, NUM_HEAD, NK]
        ),
        op=ALU.mult,
    )
    score_v = score.rearrange("p (c h k) -> p c h k", c=CH, h=NUM_HEAD)

    def score_b3(k):
        s = score_v[:, :, :, k].rearrange("p c h -> p (c h)")
        return s.unsqueeze(2).broadcast_to([128, CH * NUM_HEAD, MID_SIZE])

    def chm(t):
        return t[:, :].rearrange("p (q m) -> p q m", m=MID_SIZE)

    ph = [psPH.tile([128, 512], F32, tag="ph", name=f"ph{k}") for k in range(NK)]
    nh_lhs = [
        lambda c: xtAB[64:128, 128 * c : 128 * (c + 1)],
        lambda c: xtAB[0:64, 512 + 128 * c : 640 + 128 * c],
        lambda c: xtAB[64:128, 512 + 128 * c : 640 + 128 * c],
        lambda c: xtC[0:64, 128 * c : 128 * (c + 1)],
    ]
    nh_w = [w_h_bot, w_h_top, w_h_bot, w_h_top]
    nh_tp = [(64, 0), (0, 0), (64, 0), (0, 0)]
    for c in range(CH):
        for k in (0, 1):
            nc.tensor.matmul(
                ph[k][:, 128 * c : 128 * (c + 1)], nh_lhs[k](c), nh_w[k],
                tile_position=nh_tp[k],
            )
    for c in range(CH):
        for k in (2, 3):
            nc.tensor.matmul(
                ph[k][:, 128 * c : 128 * (c + 1)], nh_lhs[k](c), nh_w[k],
                tile_position=nh_tp[k],
            )

    wk01 = sbuf.tile([128, 1024], BF16, tag="wk01")
    wk23 = sbuf.tile([128, 1024], BF16, tag="wk23")
    for k in (0, 1):
        ph_r = sbuf.tile([128, 512], BF16, tag=f"ph_r{k}", name=f"ph_r{k}")
        nc.scalar.activation(ph_r[:, :], ph[k][:, :], AF.Relu)
        nc.gpsimd.tensor_tensor(
            chm(wk01[:, 512 * k : 512 * (k + 1)]), chm(ph_r), score_b3(k), op=ALU.mult
        )
    for k in (2, 3):
        nc.vector.scalar_tensor_tensor(
            chm(wk23[:, 512 * (k - 2) : 512 * (k - 1)]), chm(ph[k]), 0.0,
            score_b3(k), op0=ALU.max, op1=ALU.mult,
        )

    u01 = sbuf.tile([128, 512], BF16, tag="u01")
    u23 = sbuf.tile([128, 512], BF16, tag="u23")
    outacc = sbuf.tile([128, 512], BF16, tag="outacc")
    nc.vector.tensor_tensor(u01[:, :], wk01[:, 0:512], wk01[:, 512:1024], op=ALU.add)
    nc.gpsimd.tensor_tensor(u23[:, :], wk23[:, 0:512], wk23[:, 512:1024], op=ALU.add)
    nc.vector.tensor_tensor(outacc[:, :], u01[:, :], u23[:, :], op=ALU.add)

    for c in range(CH):
        nc.tensor.transpose(
            oT_ps[:, 128 * c : 128 * (c + 1)], outacc[:, 128 * c : 128 * (c + 1)],
            identb[:, :],
        )
    oT = sbuf.tile([128, 512], BF16, tag="oTsb")
    nc.scalar.activation(oT[:, :], oT_ps[:, :], AF.Copy)
    y_ps = psTX.tile([128, 256], F32, tag="tx", name="y_ps")
    for c in range(CH):
        nc.tensor.matmul(
            y_ps[:, 64 * c : 64 * (c + 1)], oT[:, 128 * c : 128 * (c + 1)], wo4[:, :]
        )
    y_sb = sbuf.tile([128, 256], F32, tag="y_sb")
    nc.scalar.activation(y_sb[:, :], y_ps[:, :], AF.Relu)
    nc.sync.dma_start(
        y[b0 : b0 + NA, :].rearrange("(c p) f -> p c f", p=128), y_sb[:, :]
    )


def _build_v4(n_pad):
    key = ("v4", n_pad)
    if key in _BUILD_CACHE:
        return _BUILD_CACHE[key]
    assert n_pad % 512 == 0
    nc = bacc.Bacc()
    xt = nc.dram_tensor("xt", [320, n_pad], BF16, kind="ExternalInput").ap()
    mask = nc.dram_tensor("mask", [n_pad, NK], I32, kind="ExternalInput").ap()
    wstk_d = nc.dram_tensor("wstk", [128, 384], BF16, kind="ExternalInput").ap()
    hsel_d = nc.dram_tensor("hsel", [128, 128], BF16, kind="ExternalInput").ap()
    wo4_d = nc.dram_tensor("wo4", [HM, OUT], BF16, kind="ExternalInput").ap()
    y = nc.dram_tensor("y", [n_pad, OUT], F32, kind="ExternalOutput").ap()

    with ExitStack() as ctx:
        tc = ctx.enter_context(tile.TileContext(nc))
        const = ctx.enter_context(tc.tile_pool(name="const", bufs=1))
        inp = ctx.enter_context(tc.tile_pool(name="inp", bufs=3))
        sbuf = ctx.enter_context(tc.tile_pool(name="sbuf", bufs=2))
        psPR = ctx.enter_context(tc.tile_pool(name="psPR", bufs=3, space="PSUM"))
        psPH = ctx.enter_context(tc.tile_pool(name="psPH", bufs=2, space="PSUM"))
        psA = ctx.enter_context(tc.tile_pool(name="psA", bufs=1, space="PSUM"))
        psTX = ctx.enter_context(tc.tile_pool(name="psTX", bufs=2, space="PSUM"))

        wstk = const.tile([128, 384], BF16)
        nc.sync.dma_start(wstk[:], wstk_d[:, :])
        hsel4 = const.tile([128, 128], BF16)
        nc.sync.dma_start(hsel4[:], hsel_d[:, :])
        wo4 = const.tile([HM, OUT], BF16)
        nc.sync.dma_start(wo4[:], wo4_d[:, :])
        identb = const.tile([128, 128], BF16)
        masks.make_identity(nc, identb[:])

        pools = (inp, sbuf, psPR, psPH, psA, psTX)
        aps = (xt, mask, y, wstk, hsel4, wo4, identb)
        for b in range(n_pad // 512):
            _emit_block_v4(tc, pools, aps, b * 512)

    nc.compile()
    _BUILD_CACHE[key] = nc
    return nc


def _build_v2(n_pad, stage=99):
    key = ("v2", n_pad, stage)
    if key in _BUILD_CACHE:
        return _BUILD_CACHE[key]
    assert n_pad % 512 == 0
    nc = bacc.Bacc()
    agent = nc.dram_tensor("agent", [n_pad, AGENT_SIZE], F32, kind="ExternalInput").ap()
    neigh = nc.dram_tensor(
        "neighbor", [n_pad, NK * NEIGH_SIZE], F32, kind="ExternalInput"
    ).ap()
    mask = nc.dram_tensor("mask", [n_pad, NK], I32, kind="ExternalInput").ap()
    wst_d = nc.dram_tensor("wst", [128, 384], BF16, kind="ExternalInput").ap()
    hsel_d = nc.dram_tensor("hsel", [128, 128], BF16, kind="ExternalInput").ap()
    wo4_d = nc.dram_tensor("wo4", [HM, OUT], BF16, kind="ExternalInput").ap()
    y = nc.dram_tensor("y", [n_pad, OUT], F32, kind="ExternalOutput").ap()

    with ExitStack() as ctx:
        tc = ctx.enter_context(tile.TileContext(nc))
        const = ctx.enter_context(tc.tile_pool(name="const", bufs=1))
        inp = ctx.enter_context(tc.tile_pool(name="inp", bufs=3))
        xtp = ctx.enter_context(tc.tile_pool(name="xtp", bufs=2))
        sbuf = ctx.enter_context(tc.tile_pool(name="sbuf", bufs=2))
        psA = ctx.enter_context(tc.tile_pool(name="psA", bufs=4, space="PSUM"))
        psT = ctx.enter_context(tc.tile_pool(name="psT", bufs=1, space="PSUM"))
        psS = ctx.enter_context(tc.tile_pool(name="psS", bufs=3, space="PSUM"))

        wst = const.tile([128, 384], BF16)
        nc.sync.dma_start(wst[:], wst_d[:, :])
        hsel4 = const.tile([128, 128], BF16)
        nc.sync.dma_start(hsel4[:], hsel_d[:, :])
        wo4 = const.tile([HM, OUT], BF16)
        nc.sync.dma_start(wo4[:], wo4_d[:, :])
        identb = const.tile([128, 128], BF16)
        masks.make_identity(nc, identb[:])

        pools = (inp, xtp, sbuf, psA, psT, psS)
        aps = (agent, neigh, mask, y, wst, hsel4, wo4, identb, None)
        for b in range(n_pad // 512):
            _emit_block_v2(tc, pools, aps, b * 512, False, stage=stage)

    nc.compile()
    _BUILD_CACHE[key] = nc
    return nc


def kernel(
    agent,
    neighbor,
    neighbor_mask,
    W_agent,
    b_agent,
    W_neigh,
    b_neigh,
    W_hid,
    b_hid,
    W_out,
    b_out,
    _trace=False,
):
    n = agent.shape[0]
    assert n % N_CORES == 0
    npc = n // N_CORES

    agent = np.ascontiguousarray(np.asarray(agent, dtype=np.float32))
    neighbor = np.ascontiguousarray(np.asarray(neighbor, dtype=np.float32)).reshape(n, NK * NEIGH_SIZE)
    neighbor_mask = np.ascontiguousarray(np.asarray(neighbor_mask, dtype=np.int32))

    biases = np.concatenate(
        [
            np.asarray(b_agent, np.float32).ravel(),
            np.asarray(b_neigh, np.float32).ravel(),
            np.asarray(b_hid, np.float32).ravel(),
            np.asarray(b_out, np.float32).ravel(),
        ]
    )[None, :]
    has_bias = bool(np.any(biases))
    kver = os.environ.get("GAT_KERNEL_VER", "4")
    use_v4 = (not has_bias) and kver == "4"
    use_v2 = (not has_bias) and not use_v4 and kver != "1"

    if use_v4:
        import ml_dtypes

        bf16 = ml_dtypes.bfloat16
        npad = ((npc + 511) // 512) * 512
        nc = _build_v4(npad)
        wa = np.asarray(W_agent, np.float32)
        wn = np.asarray(W_neigh, np.float32)
        wh = np.asarray(W_hid, np.float32)
        wstk = np.concatenate(
            [
                np.concatenate([wa, wn], axis=0),
                np.concatenate([wn, wh], axis=0),
                np.concatenate([wh, wh], axis=0),
            ],
            axis=1,
        ).astype(bf16)
        hsel = np.zeros((128, 128), np.float32)
        for j in range(4):
            for h in range(4):
                hsel[h * 32 : (h + 1) * 32, 32 * j + h] = 1.0
        wmaps = {
            "wstk": wstk,
            "hsel": hsel.astype(bf16),
            "wo4": (np.asarray(W_out, np.float32) / 4.0).astype(bf16),
        }
        Xbf = np.concatenate([agent, neighbor], axis=1).astype(bf16)
        pad = npad - npc
        in_maps = []
        for i in range(N_CORES):
            sl = slice(i * npc, (i + 1) * npc)
            xt_c = np.zeros((320, npad), bf16)
            xt_c[:, :npc] = Xbf[sl].T
            m = {
                "xt": xt_c,
                "mask": np.pad(neighbor_mask[sl], ((0, pad), (0, 0))),
                **wmaps,
            }
            in_maps.append(m)
        res = run_bass_kernel_spmd(nc, in_maps, list(range(N_CORES)), trace=_trace)
        out = np.concatenate(
            [res.results[i]["y"][:npc] for i in range(N_CORES)], axis=0
        )
        if _trace:
            kernel._last_results = res
        return out

    if use_v2:
        import ml_dtypes

        bf16 = ml_dtypes.bfloat16
        npad = ((npc + 511) // 512) * 512
        nc = _build_v2(npad)
        wa = np.asarray(W_agent, np.float32)
        wn = np.asarray(W_neigh, np.float32)
        wh = np.asarray(W_hid, np.float32)
        wst = np.concatenate(
            [
                np.concatenate([wa, wa], axis=0),
                np.concatenate([wn, wn], axis=0),
                np.concatenate([wh, wh], axis=0),
            ],
            axis=1,
        ).astype(bf16)
        hsel = np.zeros((128, 128), np.float32)
        for j in range(4):
            for h in range(4):
                hsel[h * 32 : (h + 1) * 32, 32 * j + h] = 1.0
        wmaps = {
            "wst": wst,
            "hsel": hsel.astype(bf16),
            "wo4": (np.asarray(W_out, np.float32) / 4.0).astype(bf16),
        }
        pad = npad - npc
        in_maps = []
        for i in range(N_CORES):
            sl = slice(i * npc, (i + 1) * npc)
            m = {
                "agent": np.pad(agent[sl], ((0, pad), (0, 0))),
                "neighbor": np.pad(neighbor[sl], ((0, pad), (0, 0))),
                "mask": np.pad(neighbor_mask[sl], ((0, pad), (0, 0))),
                **wmaps,
            }
            in_maps.append(m)
        res = run_bass_kernel_spmd(nc, in_maps, list(range(N_CORES)), trace=_trace)
        out = np.concatenate(
            [res.results[i]["y"][:npc] for i in range(N_CORES)], axis=0
        )
        if _trace:
            kernel._last_results = res
        return out

    nc = _build(npc, has_bias)

    wmaps = {
        "wa": np.asarray(W_agent, np.float32),
        "wn": np.asarray(W_neigh, np.float32),
        "wh": np.asarray(W_hid, np.float32),
        "wo4": np.asarray(W_out, np.float32) / 4.0,
        "biases": biases.astype(np.float32),
    }
    in_maps = []
    for i in range(N_CORES):
        sl = slice(i * npc, (i + 1) * npc)
        in_maps.append(
            {
                "agent": agent[sl],
                "neighbor": neighbor[sl],
                "mask": neighbor_mask[sl],
                **wmaps,
            }
        )

    res = run_bass_kernel_spmd(nc, in_maps, list(range(N_CORES)), trace=_trace)
    out = np.concatenate([res.results[i]["y"] for i in range(N_CORES)], axis=0)
    if _trace:
        kernel._last_results = res
    return out

